# revision 14
# baseline (speedup 1.0000x reference)
"""Trainium2 Bass kernel for 2-layer GAT (nn_GAT_86535001080291).

Strategy (dst-sharded graph parallelism over 8 NeuronCores):
  - Core r owns destination nodes [r*12500, (r+1)*12500).
  - Per-node "table" rows (256B, bf16) hold per-node quantities:
      layer1: [h0(32) | 1 | h1(32) | 1 | s0 s1 d0 d1]
      layer2: [h2(32) | 1 | s2 | d2]
    Each core builds its own table shard from its x shard; shards are
    replicated via AllGather.
  - Edges (self-loops handled separately) are grouped per core into
    supertiles of 128 dst nodes; per (supertile, src-bucket) runs are
    padded to 128-edge chunks (buckets of <=25088 table rows keep
    dma_gather's int16 indices in range). Per-edge source rows are
    fetched with nc.gpsimd.dma_gather. Self-loop rows come from an
    SBUF-resident copy of the core's own table shard (one 128-row
    "self chunk" per supertile; no gather, no bucket inflation).
  - One-hot structure matrices are built ON DEVICE from a compact
    per-slot dst-local-id array (dval, bf16):
      m0 [slot, dst]  = (iota_row == dval_col)   for aggregation
      m0t [dst, slot] = (iota_col == dval_bc)    for d-expansion
    This keeps host->device input traffic tiny (the measured harness
    cost is dominated by per-iteration input shipping).
      t = s_src + d_dst        : d-expansion via PE matmul m0t.T @ d (+ s)
      ex = exp(leakyrelu(t))   : DVE + ACT (logits bounded -> exp safe)
      messages *= ex           : DVE broadcast multiply (incl. ones col)
      agg[dst]  = m0.T @ msgs  : PE matmul accumulating in PSUM; the
                                 ones-column yields the softmax denom
      out[dst]  = agg / denom  (+ bias, gelu between layers)
All host-side preprocessing depends only on edge_index (graph structure).
"""
import math
from dataclasses import dataclass

import numpy as np
import ml_dtypes

import concourse.bacc as bacc
import concourse.mybir as mybir
from concourse.tile import TileContext
from concourse.masks import make_identity
from concourse.tile_rust import add_dep_helper
from concourse import library_config

F32 = mybir.dt.float32
BF16 = mybir.dt.bfloat16
FP8 = mybir.dt.float8e4
I16 = mybir.dt.int16
I32 = mybir.dt.int32
P = 128


@dataclass
class Cfg:
    N: int = 100000
    E: int = 1600000
    IN: int = 64
    HID: int = 32
    HEADS: int = 2
    OUT: int = 32
    neg: float = 0.2
    CORES: int = 8
    GST: int = 4             # supertiles per group
    BUCKET_ROWS: int = 25088  # int16 gather index limit
    sim_gelu: bool = False   # tanh-approx gelu (CoreSim lacks Gelu LUT)

    @property
    def SHARD(self):
        return self.N // self.CORES

    @property
    def ST(self):
        return math.ceil(self.SHARD / P)

    @property
    def SHARD_PAD(self):
        return self.ST * P

    @property
    def NT(self):
        return self.CORES * self.SHARD_PAD

    @property
    def NBUCK(self):
        return max(1, math.ceil(self.NT / self.BUCKET_ROWS))

    @property
    def BROWS(self):
        return (self.NT + self.NBUCK - 1) // self.NBUCK


def build_schedule(cfg, B_sb):
    """Shared (core-independent) static schedule from padded chunk counts.

    Per group: slot layout is bucket-major over the group's supertiles,
    followed by one full 128-slot self chunk per supertile.
    """
    groups = []
    ch0g = 0
    st = 0
    while st < cfg.ST:
        sts = list(range(st, min(st + cfg.GST, cfg.ST)))
        g = dict(sts=sts, ch0g=ch0g)
        ci = 0
        calls = []
        runs = {}
        for b in range(cfg.NBUCK):
            off16 = ci * 8
            ch0b = ci
            for s in sts:
                B = int(B_sb[s][b])
                runs[(s, b)] = (ci, B)
                ci += B
            calls.append((off16, (ci - ch0b) * P, ch0b))
        g["nb"] = ci                      # bucket chunks
        g["self_ci"] = {s: ci + i for i, s in enumerate(sts)}
        ci += len(sts)
        g["nchg"] = ci
        g["calls"] = calls
        g["runs"] = runs
        # chunk -> st map (self chunks included)
        c2s = np.zeros(ci, dtype=np.int64)
        for (s, b), (c0, B) in runs.items():
            c2s[c0:c0 + B] = s
        for s, c in g["self_ci"].items():
            c2s[c] = s
        g["c2s"] = c2s
        # st-major chunk order for aggregation (self chunk last)
        order = {}
        for s in sts:
            lst = []
            for b in range(cfg.NBUCK):
                c0, B = runs[(s, b)]
                lst.extend(range(c0, c0 + B))
            lst.append(g["self_ci"][s])
            order[s] = lst
        g["st_chunks"] = order
        ch0g += ci
        groups.append(g)
        st += cfg.GST
    nch_total = ch0g
    nch_pad = (nch_total + 15) // 16 * 16
    return groups, nch_total, nch_pad


def preprocess(edge_index, cfg):
    """Pure graph preprocessing: per-core gather indices + dst-id slots."""
    src = edge_index[0].astype(np.int64)
    dst = edge_index[1].astype(np.int64)

    SH, SP, ST, NB, BR = cfg.SHARD, cfg.SHARD_PAD, cfg.ST, cfg.NBUCK, cfg.BROWS

    per_core = []
    cnt = np.zeros((cfg.CORES, ST, NB), dtype=np.int64)
    for r in range(cfg.CORES):
        m = (dst >= r * SH) & (dst < (r + 1) * SH)
        s_r = src[m]
        d_r = dst[m] - r * SH
        srow = (s_r // SH) * SP + (s_r % SH)
        b_r = srow // BR
        st_r = d_r // P
        per_core.append((srow, d_r, b_r, st_r))
        np.add.at(cnt[r], (st_r, b_r), 1)

    B_sb = np.ceil(cnt.max(axis=0) / P).astype(np.int64)  # [ST, NB]
    groups, nch_total, nch_pad = build_schedule(cfg, B_sb)
    NG = len(groups)
    CHmax = max(g["nchg"] for g in groups)
    C16max = max(g["nb"] for g in groups) * 8

    # lookup tables: (st, b) -> group idx, group-relative chunk offset
    gi_tab = np.zeros((ST, NB), np.int64)
    ch0_tab = np.zeros((ST, NB), np.int64)
    ch0g_tab = np.zeros(ST, np.int64)
    for gi, g in enumerate(groups):
        for (s, b), (c0, B) in g["runs"].items():
            gi_tab[s, b] = gi
            ch0_tab[s, b] = c0
        for s in g["sts"]:
            ch0g_tab[s] = g["ch0g"]

    arrays = []
    for r in range(cfg.CORES):
        srow, d_r, b_r, st_r = per_core[r]
        # sort edges by (st, b) cell, compute rank within cell
        cell_key = st_r * NB + b_r
        order = np.argsort(cell_key, kind="stable")
        srow, d_r, b_r, st_r = srow[order], d_r[order], b_r[order], st_r[order]
        cell_key = cell_key[order]
        ne = len(cell_key)
        if ne:
            change = np.empty(ne, dtype=bool)
            change[0] = True
            change[1:] = cell_key[1:] != cell_key[:-1]
            starts = np.flatnonzero(change)
            rank = np.arange(ne) - np.repeat(
                starts, np.diff(np.append(starts, ne)))
        else:
            rank = np.zeros(0, np.int64)

        gi_e = gi_tab[st_r, b_r]
        slot = ch0_tab[st_r, b_r] * P + rank     # group-relative slot

        idx16 = np.zeros((NG, 16, C16max), dtype=np.int16)
        idx16[gi_e, slot % 16, slot // 16] = (srow - b_r * BR).astype(np.int16)

        dval8 = np.full((nch_pad, P), 255, dtype=np.uint8)
        gslot = (ch0g_tab[st_r] * P + slot)      # global slot
        dval8[gslot // P, gslot % P] = (d_r % P).astype(np.uint8)
        # self chunks: dloc = iota
        for g in groups:
            for s, c in g["self_ci"].items():
                dval8[g["ch0g"] + c, :] = np.arange(P, dtype=np.uint8)

        arrays.append(dict(
            idx=idx16,
            dval=dval8.astype(ml_dtypes.bfloat16),
        ))

    sched = dict(groups=groups, NG=NG, CHmax=CHmax, C16max=C16max,
                 nch_total=nch_total, nch_pad=nch_pad)
    return sched, arrays


def build_nc(cfg, sched):
    nc = bacc.Bacc("TRN2", target_bir_lowering=False, num_swdge_queues=4)
    NG, CHmax, C16max = sched["NG"], sched["CHmax"], sched["C16max"]
    nch_pad = sched["nch_pad"]
    groups = sched["groups"]
    ST, NT, SP, NB, BR = cfg.ST, cfg.NT, cfg.SHARD_PAD, cfg.NBUCK, cfg.BROWS
    H = cfg.HEADS

    # ---- external I/O ----
    x_ext = nc.dram_tensor("x", [SP, cfg.IN], BF16, kind="ExternalInput")
    W1_ext = nc.dram_tensor("W1", [cfg.IN, H * cfg.HID], F32, kind="ExternalInput")
    as1_ext = nc.dram_tensor("a_src1", [H, cfg.HID], F32, kind="ExternalInput")
    ad1_ext = nc.dram_tensor("a_dst1", [H, cfg.HID], F32, kind="ExternalInput")
    b1_ext = nc.dram_tensor("b1", [1, H * cfg.HID], F32, kind="ExternalInput")
    W2_ext = nc.dram_tensor("W2", [H * cfg.HID, cfg.OUT], F32, kind="ExternalInput")
    as2_ext = nc.dram_tensor("a_src2", [1, cfg.OUT], F32, kind="ExternalInput")
    ad2_ext = nc.dram_tensor("a_dst2", [1, cfg.OUT], F32, kind="ExternalInput")
    b2_ext = nc.dram_tensor("b2", [1, cfg.OUT], F32, kind="ExternalInput")
    idx_ext = nc.dram_tensor("idx", [NG, 16, C16max], I16, kind="ExternalInput")
    dval_ext = nc.dram_tensor("dval", [nch_pad, P], BF16, kind="ExternalInput")
    out_ext = nc.dram_tensor("out", [cfg.SHARD, cfg.OUT], BF16, kind="ExternalOutput")

    SD1 = 2 * (cfg.HID + 1)      # layer1 s-col offset (=66)
    NC1 = SD1 + 2 * H            # layer1 packed width (=70)
    NC2 = cfg.OUT + 2            # layer2 matmul width (h|s|d)

    with TileContext(nc) as tc:
        with (
            tc.tile_pool(name="dram", bufs=1, space="DRAM") as dpool,
            tc.tile_pool(name="const", bufs=1) as cpool,
            tc.tile_pool(name="work", bufs=3) as wpool,
            tc.tile_pool(name="gath", bufs=2) as gpool,
            tc.tile_pool(name="bc", bufs=1) as bcpool,
        ):
            nc.gpsimd.load_library(library_config.mlp)

            t1_shard = dpool.tile([SP, P], BF16)
            t1_full = dpool.tile([NT, P], BF16, addr_space="Shared")
            t2_shard = dpool.tile([SP, P], BF16)
            t2_full = dpool.tile([NT, P], BF16, addr_space="Shared")

            ident = cpool.tile([P, P], F32)
            make_identity(nc, ident[:])
            identb = cpool.tile([P, P], BF16)
            nc.vector.tensor_copy(out=identb[:], in_=ident[:])
            io32 = cpool.tile([P, P], I32)
            nc.gpsimd.iota(io32[:], pattern=[[1, P]], base=0, channel_multiplier=0)
            iorow = cpool.tile([P, P], BF16)
            nc.vector.tensor_copy(out=iorow[:], in_=io32[:])
            ic32 = cpool.tile([P, 1], I32)
            nc.gpsimd.iota(ic32[:], pattern=[[0, 1]], base=0, channel_multiplier=1)
            iocol = cpool.tile([P, 1], BF16)
            nc.vector.tensor_copy(out=iocol[:], in_=ic32[:])

            # per-slot dst-local ids, column layout [slot-in-chunk, chunk]
            dcol_all = cpool.tile([P, nch_pad], BF16)
            nc.sync.dma_start(out=dcol_all[:], in_=dval_ext[:, :], transpose=True)

            _pp0cm = tc.tile_pool(name="psum0", bufs=2, space="PSUM")
            ppool0 = _pp0cm.__enter__()

            # ---------- weight prep ----------
            w1_t = cpool.tile([cfg.IN, H * cfg.HID], F32)
            nc.sync.dma_start(out=w1_t[:], in_=W1_ext[:, :])
            w2_t = cpool.tile([H * cfg.HID, cfg.OUT], F32)
            nc.sync.dma_start(out=w2_t[:], in_=W2_ext[:, :])
            av = cpool.tile([cfg.HID, 2 * H + 2], F32)
            for h in range(H):
                nc.sync.dma_start(out=av[:, h:h + 1], in_=as1_ext[h:h + 1, :])
                nc.sync.dma_start(out=av[:, H + h:H + h + 1], in_=ad1_ext[h:h + 1, :])
            nc.sync.dma_start(out=av[:, 2 * H:2 * H + 1], in_=as2_ext[0:1, :])
            nc.sync.dma_start(out=av[:, 2 * H + 1:2 * H + 2], in_=ad2_ext[0:1, :])

            w1Th = cpool.tile([cfg.HID, H, cfg.IN], F32)
            for h in range(H):
                w1Th_p = ppool0.tile([cfg.HID, cfg.IN], F32, space="PSUM", tag="prep")
                nc.tensor.transpose(out=w1Th_p[:],
                                    in_=w1_t[:, h * cfg.HID:(h + 1) * cfg.HID],
                                    identity=ident[0:cfg.IN, 0:cfg.IN])
                nc.vector.tensor_copy(out=w1Th[:, h, :], in_=w1Th_p[:])
            w2T_p = ppool0.tile([cfg.OUT, H * cfg.HID], F32, space="PSUM", tag="prep")
            nc.tensor.transpose(out=w2T_p[:], in_=w2_t[:, :],
                                identity=ident[0:H * cfg.HID, 0:H * cfg.HID])
            w2T = cpool.tile([cfg.OUT, H * cfg.HID], F32)
            nc.vector.tensor_copy(out=w2T[:], in_=w2T_p[:])

            wv_p = ppool0.tile([cfg.IN, 2 * H + 2], F32, space="PSUM", tag="prep2")
            for h in range(H):
                nc.tensor.matmul(out=wv_p[:, h:h + 1],
                                 lhsT=w1Th[:, h, :],
                                 rhs=av[0:cfg.HID, h:h + 1], start=True, stop=True)
                nc.tensor.matmul(out=wv_p[:, H + h:H + h + 1],
                                 lhsT=w1Th[:, h, :],
                                 rhs=av[0:cfg.HID, H + h:H + h + 1], start=True, stop=True)
            nc.tensor.matmul(out=wv_p[0:H * cfg.HID, 2 * H:2 * H + 1], lhsT=w2T[:, :],
                             rhs=av[0:cfg.OUT, 2 * H:2 * H + 1], start=True, stop=True)
            nc.tensor.matmul(out=wv_p[0:H * cfg.HID, 2 * H + 1:2 * H + 2], lhsT=w2T[:, :],
                             rhs=av[0:cfg.OUT, 2 * H + 1:2 * H + 2], start=True, stop=True)

            # W1ext bf16 [IN, 70]: [W1h0 | 0 | W1h1 | 0 | s0 s1 d0 d1]
            w1e = cpool.tile([cfg.IN, NC1], BF16)
            for h in range(H):
                nc.vector.tensor_copy(out=w1e[:, h * (cfg.HID + 1):h * (cfg.HID + 1) + cfg.HID],
                                      in_=w1_t[:, h * cfg.HID:(h + 1) * cfg.HID])
                nc.vector.memset(w1e[:, h * (cfg.HID + 1) + cfg.HID:(h + 1) * (cfg.HID + 1)], 0.0)
            nc.vector.tensor_copy(out=w1e[:, SD1:SD1 + H], in_=wv_p[:, 0:H])
            nc.vector.tensor_copy(out=w1e[:, SD1 + H:NC1], in_=wv_p[:, H:2 * H])
            # W2ext f32 [64, 34]: [W2 | s2vec | d2vec]
            w2e = cpool.tile([H * cfg.HID, NC2], F32)
            nc.vector.tensor_copy(out=w2e[:, 0:cfg.OUT], in_=w2_t[:, :])
            nc.vector.tensor_copy(out=w2e[:, cfg.OUT:NC2],
                                  in_=wv_p[0:H * cfg.HID, 2 * H:2 * H + 2])

            b1_bc = cpool.tile([P, H, cfg.HID], F32)
            b1_row = cpool.tile([1, H * cfg.HID], F32)
            nc.sync.dma_start(out=b1_row[:], in_=b1_ext[:, :])
            nc.gpsimd.partition_broadcast(
                out_ap=b1_bc[:].rearrange("p h d -> p (h d)"), in_ap=b1_row[:])
            b2_bc = cpool.tile([P, cfg.OUT], F32)
            b2_row = cpool.tile([1, cfg.OUT], F32)
            nc.sync.dma_start(out=b2_row[:], in_=b2_ext[:, :])
            nc.gpsimd.partition_broadcast(out_ap=b2_bc[:], in_ap=b2_row[:])

            # ---------- persistent per-node state ----------
            own1 = cpool.tile([P, ST, P], BF16)    # own-shard table1 rows
            own2 = cpool.tile([P, ST, P], BF16)    # own-shard table2 rows
            g_all = cpool.tile([P, ST, H, cfg.HID], F32)
            d1o = cpool.tile([P, ST, H], BF16)
            d2o = cpool.tile([P, ST, 1], BF16)
            nc.vector.memset(own1[:].rearrange("p a b -> p (a b)"), 0.0)
            nc.vector.memset(own2[:].rearrange("p a b -> p (a b)"), 0.0)

            # ---------- phase 1: table1 shard build ----------
            t1_writes = []
            for st in range(ST):
                xf = wpool.tile([P, cfg.IN], BF16, tag="xf")
                nc.sync.dma_start(out=xf[:], in_=x_ext[st * P:(st + 1) * P, :])
                xT_p = ppool0.tile([cfg.IN, P], BF16, space="PSUM", tag="xT")
                nc.tensor.transpose(out=xT_p[:], in_=xf[:], identity=identb[:])
                xT = wpool.tile([cfg.IN, P], BF16, tag="xTs")
                nc.vector.tensor_copy(out=xT[:], in_=xT_p[:])
                hp = ppool0.tile([P, NC1], F32, space="PSUM", tag="hp")
                nc.tensor.matmul(out=hp[:, :], lhsT=xT[:], rhs=w1e[:, :],
                                 start=True, stop=True)
                nc.vector.tensor_copy(out=own1[:, st, 0:NC1], in_=hp[:, :])
                ones_view = own1[:, st, 0:SD1].rearrange(
                    "p (h d) -> p h d", h=H)[:, :, cfg.HID:cfg.HID + 1]
                nc.vector.memset(ones_view, 1.0)
                nc.vector.tensor_copy(out=d1o[:, st, :], in_=hp[:, SD1 + H:NC1])
                t1_writes.append(nc.sync.dma_start(
                    out=t1_shard[st * P:(st + 1) * P, :], in_=own1[:, st, :]).ins)

            cc1 = nc.gpsimd.collective_compute(
                "AllGather", mybir.AluOpType.bypass,
                ins=[t1_shard[:].opt()], outs=[t1_full[:].opt()],
                replica_groups=[list(range(cfg.CORES))])
            for w in t1_writes:
                add_dep_helper(cc1.ins, w, reason="t1 shard complete before AG")
            fences = {1: cc1.ins}

            _pp0cm.__exit__(None, None, None)
            _pp1cm = tc.tile_pool(name="psum_e", bufs=2, space="PSUM")
            ppool1 = _pp1cm.__enter__()
            _pp2cm = tc.tile_pool(name="psum_t", bufs=1, space="PSUM")
            ppool2 = _pp2cm.__enter__()

            # ---------- shared edge-pass ----------
            qn_counter = [0]

            def edge_pass(layer):
                if layer == 1:
                    table, heads, scol, own = t1_full, H, SD1, own1
                    mw = cfg.HID + 1
                    down = d1o
                else:
                    table, heads, scol, own = t2_full, 1, cfg.OUT + 1, own2
                    mw = cfg.OUT + 1
                    down = d2o
                for gi, g in enumerate(groups):
                    nchg, nb, ch0g = g["nchg"], g["nb"], g["ch0g"]
                    sts = g["sts"]
                    ns = len(sts)
                    S = nchg * P
                    # structure build
                    idx_t = gpool.tile([P, C16max], I16, tag="idx")
                    if nb:
                        for k in range(8):
                            nc.sync.dma_start(
                                out=idx_t[16 * k:16 * (k + 1), 0:nb * 8],
                                in_=idx_ext[gi, :, 0:nb * 8])
                    dbc = bcpool.tile([P, CHmax * P], BF16, tag="dbc")
                    nc.sync.dma_start(
                        out=dbc[:, 0:S],
                        in_=dval_ext[ch0g:ch0g + nchg, :]
                        .rearrange("a b -> (a b)")
                        .rearrange("(a s) -> a s", a=1).to_broadcast([P, S]))
                    m0 = gpool.tile([P, CHmax, P], FP8, tag="m0")
                    nc.vector.tensor_tensor(
                        out=m0[:, 0:nchg, :],
                        in0=dcol_all[:, ch0g:ch0g + nchg]
                        .rearrange("p (c a) -> p c a", a=1).to_broadcast([P, nchg, P]),
                        in1=iorow[:].rearrange("p (a b) -> p a b", a=1)
                        .to_broadcast([P, nchg, P]),
                        op=mybir.AluOpType.is_equal)
                    m0t = gpool.tile([P, CHmax * P], FP8, tag="m0t")
                    nc.vector.tensor_tensor(
                        out=m0t[:, 0:S], in0=dbc[:, 0:S],
                        in1=iocol[:].to_broadcast([P, S]),
                        op=mybir.AluOpType.is_equal)
                    # gather + self rows
                    gath = gpool.tile([P, CHmax, P], BF16, tag="gath")
                    for b in range(NB):
                        off16, nidx, ch0 = g["calls"][b]
                        while nidx > 0:
                            n = min(nidx, 4096)
                            qn = qn_counter[0] % 4
                            qn_counter[0] += 1
                            gi_inst = nc.gpsimd.dma_gather(
                                gath[:, ch0:ch0 + n // P, :],
                                table[b * BR:NT, :],
                                idx_t[:, off16:off16 + n // 16],
                                n, n, P, single_packet=False, queue_num=qn)
                            add_dep_helper(gi_inst.ins, fences[layer],
                                           reason="table ready before gather")
                            nidx -= n
                            ch0 += n // P
                            off16 += n // 16
                    for s in sts:
                        nc.vector.tensor_copy(
                            out=gath[:, g["self_ci"][s], :], in_=own[:, s, :])
                    # t = s_src + d_dst ; ex = exp(leakyrelu(t))
                    tp = ppool1.tile([P, CHmax, H], F32, space="PSUM", tag="tp")
                    for ci in range(nchg):
                        nc.tensor.matmul(
                            out=tp[:, ci, 0:heads],
                            lhsT=m0t[:, ci * P:(ci + 1) * P],
                            rhs=down[:, int(g["c2s"][ci]), 0:heads],
                            start=True, stop=True)
                    ts = wpool.tile([P, CHmax, H], F32, tag="ts")
                    ex = wpool.tile([P, CHmax, H], F32, tag="ex")
                    nc.vector.tensor_tensor(
                        out=ts[:, 0:nchg, 0:heads], in0=tp[:, 0:nchg, 0:heads],
                        in1=gath[:, 0:nchg, scol:scol + heads],
                        op=mybir.AluOpType.add)
                    nc.vector.tensor_scalar_mul(
                        out=ex[:, 0:nchg, 0:heads], in0=ts[:, 0:nchg, 0:heads],
                        scalar1=cfg.neg)
                    nc.vector.tensor_tensor(
                        out=ts[:, 0:nchg, 0:heads], in0=ts[:, 0:nchg, 0:heads],
                        in1=ex[:, 0:nchg, 0:heads], op=mybir.AluOpType.max)
                    nc.scalar.activation(
                        out=ex[:, 0:nchg, 0:heads], in_=ts[:, 0:nchg, 0:heads],
                        func=mybir.ActivationFunctionType.Exp)
                    for h in range(heads):
                        nc.vector.tensor_tensor(
                            out=gath[:, 0:nchg, h * mw:(h + 1) * mw],
                            in0=gath[:, 0:nchg, h * mw:(h + 1) * mw],
                            in1=ex[:, 0:nchg, h:h + 1].to_broadcast([P, nchg, mw]),
                            op=mybir.AluOpType.mult)
                    # aggregate per supertile
                    aggp = ppool1.tile([P, cfg.GST, heads, mw], F32,
                                       space="PSUM", tag="agg")
                    for sti, s in enumerate(sts):
                        lst = g["st_chunks"][s]
                        for j, ci in enumerate(lst):
                            nc.tensor.matmul(
                                out=aggp[:, sti, :, :].rearrange("p h m -> p (h m)"),
                                lhsT=m0[:, ci, :],
                                rhs=gath[:, ci, 0:heads * mw],
                                start=(j == 0), stop=(j == len(lst) - 1))
                    # normalize (+eps keeps pad rows finite)
                    rec = wpool.tile([P, cfg.GST, H, 1], F32, tag="rec")
                    nc.vector.tensor_scalar_add(
                        out=rec[:, 0:ns, 0:heads, :],
                        in0=aggp[:, 0:ns, :, mw - 1:mw], scalar1=1e-30)
                    nc.vector.reciprocal(out=rec[:, 0:ns, 0:heads, :],
                                         in_=rec[:, 0:ns, 0:heads, :])
                    st0 = sts[0]
                    if layer == 1:
                        gv = g_all[:, st0:st0 + ns, :, :]
                        nc.vector.tensor_tensor(
                            out=gv, in0=aggp[:, 0:ns, :, 0:cfg.HID],
                            in1=rec[:, 0:ns, :, :].to_broadcast([P, ns, H, cfg.HID]),
                            op=mybir.AluOpType.mult)
                        nc.vector.tensor_tensor(
                            out=gv, in0=gv,
                            in1=b1_bc[:].rearrange("p (a h) d -> p a h d", a=1)
                            .to_broadcast([P, ns, H, cfg.HID]),
                            op=mybir.AluOpType.add)
                        gvf = gv.rearrange("p a h d -> p (a h d)")
                        if cfg.sim_gelu:
                            _gelu_tanh(nc, wpool, gvf)
                        else:
                            nc.scalar.activation(
                                out=gvf, in_=gvf,
                                func=mybir.ActivationFunctionType.Gelu)
                    else:
                        ov = wpool.tile([P, cfg.GST, cfg.OUT], BF16, tag="ov")
                        nc.vector.tensor_tensor(
                            out=ov[:, 0:ns, :], in0=aggp[:, 0:ns, 0, 0:cfg.OUT],
                            in1=rec[:, 0:ns, 0, :].to_broadcast([P, ns, cfg.OUT]),
                            op=mybir.AluOpType.mult)
                        nc.vector.tensor_tensor(
                            out=ov[:, 0:ns, :], in0=ov[:, 0:ns, :],
                            in1=b2_bc[:].rearrange("p (a o) -> p a o", a=1)
                            .to_broadcast([P, ns, cfg.OUT]),
                            op=mybir.AluOpType.add)
                        for sti, s in enumerate(sts):
                            rows = min(P, cfg.SHARD - s * P)
                            nc.sync.dma_start(
                                out=out_ext[s * P:s * P + rows, :],
                                in_=ov[0:rows, sti, :])

            edge_pass(1)

            # ---------- phase: table2 shard build ----------
            t2_writes = []
            for st in range(ST):
                gT_p = ppool2.tile([H * cfg.HID, P], F32, space="PSUM", tag="gT")
                nc.tensor.transpose(
                    out=gT_p[:], in_=g_all[:, st, :, :].rearrange("p h d -> p (h d)"),
                    identity=ident[:])
                gT = wpool.tile([H * cfg.HID, P], F32, tag="gTs")
                nc.scalar.activation(out=gT[:], in_=gT_p[:],
                                     func=mybir.ActivationFunctionType.Copy)
                h2p = ppool2.tile([P, NC2], F32, space="PSUM", tag="h2p")
                nc.tensor.matmul(out=h2p[:], lhsT=gT[:], rhs=w2e[:, :],
                                 start=True, stop=True)
                nc.vector.tensor_copy(out=own2[:, st, 0:cfg.OUT], in_=h2p[:, 0:cfg.OUT])
                nc.vector.memset(own2[:, st, cfg.OUT:cfg.OUT + 1], 1.0)
                nc.vector.tensor_copy(out=own2[:, st, cfg.OUT + 1:cfg.OUT + 3],
                                      in_=h2p[:, cfg.OUT:NC2])
                nc.vector.tensor_copy(out=d2o[:, st, :], in_=h2p[:, NC2 - 1:NC2])
                t2_writes.append(nc.sync.dma_start(
                    out=t2_shard[st * P:(st + 1) * P, :], in_=own2[:, st, :]).ins)

            cc2 = nc.gpsimd.collective_compute(
                "AllGather", mybir.AluOpType.bypass,
                ins=[t2_shard[:].opt()], outs=[t2_full[:].opt()],
                replica_groups=[list(range(cfg.CORES))])
            for w in t2_writes:
                add_dep_helper(cc2.ins, w, reason="t2 shard complete before AG")
            fences[2] = cc2.ins

            edge_pass(2)
            _pp2cm.__exit__(None, None, None)
            _pp1cm.__exit__(None, None, None)

    nc.compile()
    return nc


def _gelu_tanh(nc, wpool, gv):
    """tanh-approx gelu in-place on gv [P, D] (CoreSim-compatible)."""
    Pp, D = gv.shape[0], gv.shape[1]
    t1 = wpool.tile([Pp, D], F32, tag="glu1")
    nc.scalar.activation(out=t1[:], in_=gv, scale=0.1888856,
                         func=mybir.ActivationFunctionType.Square)
    nc.vector.tensor_scalar_add(out=t1[:], in0=t1[:], scalar1=0.7978846)
    nc.vector.tensor_tensor(out=t1[:], in0=t1[:], in1=gv, op=mybir.AluOpType.mult)
    nc.scalar.activation(out=t1[:], in_=t1[:],
                         func=mybir.ActivationFunctionType.Tanh)
    nc.vector.tensor_scalar_add(out=t1[:], in0=t1[:], scalar1=1.0)
    nc.vector.tensor_tensor(out=t1[:], in0=t1[:], in1=gv, op=mybir.AluOpType.mult)
    nc.vector.tensor_scalar_mul(out=gv, in0=t1[:], scalar1=0.5)


_CACHE = {}


def _get_built(cfg, edge_index):
    key = hash((edge_index.tobytes(), cfg.N, cfg.E, cfg.GST, cfg.sim_gelu,
                cfg.BUCKET_ROWS, "v2"))
    if key not in _CACHE:
        sched, arrays = preprocess(edge_index, cfg)
        nc = build_nc(cfg, sched)
        _CACHE[key] = (nc, sched, arrays)
    return _CACHE[key]


def make_in_maps(cfg, arrays, inputs):
    x = np.ascontiguousarray(inputs["x"], dtype=np.float32)
    shared = dict(
        W1=np.ascontiguousarray(inputs["W1"], dtype=np.float32),
        a_src1=np.ascontiguousarray(inputs["a_src1"], dtype=np.float32),
        a_dst1=np.ascontiguousarray(inputs["a_dst1"], dtype=np.float32),
        b1=np.ascontiguousarray(inputs["b1"], dtype=np.float32).reshape(1, -1),
        W2=np.ascontiguousarray(inputs["W2"], dtype=np.float32),
        a_src2=np.ascontiguousarray(inputs["a_src2"], dtype=np.float32),
        a_dst2=np.ascontiguousarray(inputs["a_dst2"], dtype=np.float32),
        b2=np.ascontiguousarray(inputs["b2"], dtype=np.float32).reshape(1, -1),
    )
    in_maps = []
    for r in range(cfg.CORES):
        m = dict(shared)
        xs = np.zeros((cfg.SHARD_PAD, cfg.IN), dtype=ml_dtypes.bfloat16)
        xs[0:cfg.SHARD] = x[r * cfg.SHARD:(r + 1) * cfg.SHARD]
        m["x"] = xs
        m["idx"] = arrays[r]["idx"]
        m["dval"] = arrays[r]["dval"]
        in_maps.append(m)
    return in_maps


def kernel(x, edge_index, W1, a_src1, a_dst1, b1, W2, a_src2, a_dst2, b2,
           cfg=None, return_extras=False):
    from concourse.bass_utils import run_bass_kernel_spmd
    cfg = cfg or Cfg()
    nc, sched, arrays = _get_built(cfg, np.asarray(edge_index))
    in_maps = make_in_maps(cfg, arrays, dict(
        x=x, W1=W1, a_src1=a_src1, a_dst1=a_dst1, b1=b1,
        W2=W2, a_src2=a_src2, a_dst2=a_dst2, b2=b2))
    res = run_bass_kernel_spmd(nc, in_maps, list(range(cfg.CORES)))
    out = np.concatenate(
        [np.asarray(res.results[r]["out"]).astype(np.float32)
         for r in range(cfg.CORES)], axis=0)
    if return_extras:
        return out, res
    return out


# revision 18
# speedup vs baseline: 1.2064x; 1.2064x over previous
"""Trainium2 Bass kernel for 2-layer GAT (nn_GAT_86535001080291).

Strategy (dst-sharded graph parallelism over 8 NeuronCores):
  - Core r owns destination nodes [r*12500, (r+1)*12500).
  - Per-node "table" rows (256B, bf16) hold per-node quantities:
      layer1: [h0(32) | 1 | h1(32) | 1 | s0 s1 d0 d1]
      layer2: [h2(32) | 1 | s2 | d2]
    Each core builds its own table shard from its x shard; shards are
    replicated via AllGather.
  - Edges (self-loops handled separately) are grouped per core into
    supertiles of 128 dst nodes; per (supertile, src-bucket) runs are
    padded to 128-edge chunks (buckets of <=25088 table rows keep
    dma_gather's int16 indices in range). Per-edge source rows are
    fetched with nc.gpsimd.dma_gather. Self-loop rows come from an
    SBUF-resident copy of the core's own table shard (one 128-row
    "self chunk" per supertile; no gather, no bucket inflation).
  - One-hot structure matrices are built ON DEVICE from a compact
    per-slot dst-local-id array (dval, bf16):
      m0 [slot, dst]  = (iota_row == dval_col)   for aggregation
      m0t [dst, slot] = (iota_col == dval_bc)    for d-expansion
    This keeps host->device input traffic tiny (the measured harness
    cost is dominated by per-iteration input shipping).
      t = s_src + d_dst        : d-expansion via PE matmul m0t.T @ d (+ s)
      ex = exp(leakyrelu(t))   : DVE + ACT (logits bounded -> exp safe)
      messages *= ex           : DVE broadcast multiply (incl. ones col)
      agg[dst]  = m0.T @ msgs  : PE matmul accumulating in PSUM; the
                                 ones-column yields the softmax denom
      out[dst]  = agg / denom  (+ bias, gelu between layers)
All host-side preprocessing depends only on edge_index (graph structure).
"""
import math
from dataclasses import dataclass

import numpy as np
import ml_dtypes

import concourse.bacc as bacc
import concourse.mybir as mybir
from concourse.tile import TileContext
from concourse.masks import make_identity
from concourse.tile_rust import add_dep_helper
from concourse import library_config

F32 = mybir.dt.float32
BF16 = mybir.dt.bfloat16
FP8 = mybir.dt.float8e4
I16 = mybir.dt.int16
I32 = mybir.dt.int32
P = 128


@dataclass
class Cfg:
    N: int = 100000
    E: int = 1600000
    IN: int = 64
    HID: int = 32
    HEADS: int = 2
    OUT: int = 32
    neg: float = 0.2
    CORES: int = 8
    GST: int = 4             # supertiles per group
    BUCKET_ROWS: int = 25088  # int16 gather index limit
    sim_gelu: bool = False   # tanh-approx gelu (CoreSim lacks Gelu LUT)

    @property
    def SHARD(self):
        return self.N // self.CORES

    @property
    def ST(self):
        return math.ceil(self.SHARD / P)

    @property
    def SHARD_PAD(self):
        return self.ST * P

    @property
    def NT(self):
        return self.CORES * self.SHARD_PAD

    @property
    def NBUCK(self):
        return max(1, math.ceil(self.NT / self.BUCKET_ROWS))

    @property
    def BROWS(self):
        return (self.NT + self.NBUCK - 1) // self.NBUCK


def build_schedule(cfg, B_sb):
    """Shared (core-independent) static schedule from padded chunk counts.

    Per group: slot layout is bucket-major over the group's supertiles,
    followed by one full 128-slot self chunk per supertile.
    """
    groups = []
    ch0g = 0
    st = 0
    while st < cfg.ST:
        sts = list(range(st, min(st + cfg.GST, cfg.ST)))
        g = dict(sts=sts, ch0g=ch0g)
        ci = 0
        calls = []
        runs = {}
        for b in range(cfg.NBUCK):
            off16 = ci * 8
            ch0b = ci
            for s in sts:
                B = int(B_sb[s][b])
                runs[(s, b)] = (ci, B)
                ci += B
            calls.append((off16, (ci - ch0b) * P, ch0b))
        g["nb"] = ci                      # bucket chunks
        g["self_ci"] = {s: ci + i for i, s in enumerate(sts)}
        ci += len(sts)
        g["nchg"] = ci
        g["calls"] = calls
        g["runs"] = runs
        # chunk -> st map (self chunks included)
        c2s = np.zeros(ci, dtype=np.int64)
        for (s, b), (c0, B) in runs.items():
            c2s[c0:c0 + B] = s
        for s, c in g["self_ci"].items():
            c2s[c] = s
        g["c2s"] = c2s
        # st-major chunk order for aggregation (self chunk last)
        order = {}
        for s in sts:
            lst = []
            for b in range(cfg.NBUCK):
                c0, B = runs[(s, b)]
                lst.extend(range(c0, c0 + B))
            lst.append(g["self_ci"][s])
            order[s] = lst
        g["st_chunks"] = order
        ch0g += ci
        groups.append(g)
        st += cfg.GST
    nch_total = ch0g
    nch_pad = (nch_total + 15) // 16 * 16
    return groups, nch_total, nch_pad


def preprocess(edge_index, cfg):
    """Pure graph preprocessing: per-core gather indices + dst-id slots."""
    src = edge_index[0].astype(np.int64)
    dst = edge_index[1].astype(np.int64)

    SH, SP, ST, NB, BR = cfg.SHARD, cfg.SHARD_PAD, cfg.ST, cfg.NBUCK, cfg.BROWS

    per_core = []
    cnt = np.zeros((cfg.CORES, ST, NB), dtype=np.int64)
    for r in range(cfg.CORES):
        m = (dst >= r * SH) & (dst < (r + 1) * SH)
        s_r = src[m]
        d_r = dst[m] - r * SH
        srow = (s_r // SH) * SP + (s_r % SH)
        b_r = srow // BR
        st_r = d_r // P
        per_core.append((srow, d_r, b_r, st_r))
        np.add.at(cnt[r], (st_r, b_r), 1)

    B_sb = np.ceil(cnt.max(axis=0) / P).astype(np.int64)  # [ST, NB]
    groups, nch_total, nch_pad = build_schedule(cfg, B_sb)
    NG = len(groups)
    CHmax = max(g["nchg"] for g in groups)
    C16max = max(g["nb"] for g in groups) * 8

    # lookup tables: (st, b) -> group idx, group-relative chunk offset
    gi_tab = np.zeros((ST, NB), np.int64)
    ch0_tab = np.zeros((ST, NB), np.int64)
    ch0g_tab = np.zeros(ST, np.int64)
    for gi, g in enumerate(groups):
        for (s, b), (c0, B) in g["runs"].items():
            gi_tab[s, b] = gi
            ch0_tab[s, b] = c0
        for s in g["sts"]:
            ch0g_tab[s] = g["ch0g"]

    arrays = []
    for r in range(cfg.CORES):
        srow, d_r, b_r, st_r = per_core[r]
        # sort edges by (st, b) cell, compute rank within cell
        cell_key = st_r * NB + b_r
        order = np.argsort(cell_key, kind="stable")
        srow, d_r, b_r, st_r = srow[order], d_r[order], b_r[order], st_r[order]
        cell_key = cell_key[order]
        ne = len(cell_key)
        if ne:
            change = np.empty(ne, dtype=bool)
            change[0] = True
            change[1:] = cell_key[1:] != cell_key[:-1]
            starts = np.flatnonzero(change)
            rank = np.arange(ne) - np.repeat(
                starts, np.diff(np.append(starts, ne)))
        else:
            rank = np.zeros(0, np.int64)

        gi_e = gi_tab[st_r, b_r]
        slot = ch0_tab[st_r, b_r] * P + rank     # group-relative slot

        idx16 = np.zeros((NG, 16, C16max), dtype=np.int16)
        idx16[gi_e, slot % 16, slot // 16] = (srow - b_r * BR).astype(np.int16)

        dval8 = np.full((nch_pad, P), 255, dtype=np.uint8)
        gslot = (ch0g_tab[st_r] * P + slot)      # global slot
        dval8[gslot // P, gslot % P] = (d_r % P).astype(np.uint8)
        # self chunks: dloc = iota
        for g in groups:
            for s, c in g["self_ci"].items():
                dval8[g["ch0g"] + c, :] = np.arange(P, dtype=np.uint8)

        arrays.append(dict(
            idx=idx16,
            dval=dval8.astype(ml_dtypes.bfloat16),
        ))

    sched = dict(groups=groups, NG=NG, CHmax=CHmax, C16max=C16max,
                 nch_total=nch_total, nch_pad=nch_pad)
    return sched, arrays


def blob_layout(cfg, sched):
    """Byte layout (in int16 units) of the single per-core input blob."""
    NG, C16max = sched["NG"], sched["C16max"]
    nch_pad = sched["nch_pad"]
    H = cfg.HEADS
    off = {}
    o = 0
    off["x"] = o
    o += cfg.SHARD_PAD * cfg.IN                 # bf16
    off["dval"] = o
    o += nch_pad * P                            # bf16
    off["idx"] = o
    o += NG * 16 * C16max                       # i16
    # f32 weights (2 i16 units each)
    for name, n in [("W1", cfg.IN * H * cfg.HID), ("a_src1", H * cfg.HID),
                    ("a_dst1", H * cfg.HID), ("b1", H * cfg.HID),
                    ("W2", H * cfg.HID * cfg.OUT), ("a_src2", cfg.OUT),
                    ("a_dst2", cfg.OUT), ("b2", cfg.OUT)]:
        off[name] = o
        o += 2 * n
    return off, o


def build_nc(cfg, sched):
    nc = bacc.Bacc("TRN2", target_bir_lowering=False, num_swdge_queues=4)
    NG, CHmax, C16max = sched["NG"], sched["CHmax"], sched["C16max"]
    nch_pad = sched["nch_pad"]
    groups = sched["groups"]
    ST, NT, SP, NB, BR = cfg.ST, cfg.NT, cfg.SHARD_PAD, cfg.NBUCK, cfg.BROWS
    H = cfg.HEADS

    # ---- external I/O (single packed input blob) ----
    off, blob_len = blob_layout(cfg, sched)
    blob_ext = nc.dram_tensor("blob", [1, blob_len], I16, kind="ExternalInput")
    out_ext = nc.dram_tensor("out", [cfg.SHARD, cfg.OUT], BF16, kind="ExternalOutput")

    def bview(name, n_i16, dt, rows=None):
        v = blob_ext[0, off[name]:off[name] + n_i16]
        if dt != I16:
            v = v.bitcast(dt)
        if rows is not None:
            v = v.rearrange("(a b) -> a b", a=rows)
        return v

    x_ext = bview("x", SP * cfg.IN, BF16, rows=SP)
    dval_ext = bview("dval", nch_pad * P, BF16, rows=nch_pad)
    idx_ext = bview("idx", NG * 16 * C16max, I16).rearrange(
        "(g p c) -> g p c", g=NG, p=16)
    W1_ext = bview("W1", 2 * cfg.IN * H * cfg.HID, F32, rows=cfg.IN)
    as1_ext = bview("a_src1", 2 * H * cfg.HID, F32, rows=H)
    ad1_ext = bview("a_dst1", 2 * H * cfg.HID, F32, rows=H)
    b1_ext = bview("b1", 2 * H * cfg.HID, F32, rows=1)
    W2_ext = bview("W2", 2 * H * cfg.HID * cfg.OUT, F32, rows=H * cfg.HID)
    as2_ext = bview("a_src2", 2 * cfg.OUT, F32, rows=1)
    ad2_ext = bview("a_dst2", 2 * cfg.OUT, F32, rows=1)
    b2_ext = bview("b2", 2 * cfg.OUT, F32, rows=1)

    SD1 = 2 * (cfg.HID + 1)      # layer1 s-col offset (=66)
    NC1 = SD1 + 2 * H            # layer1 packed width (=70)
    NC2 = cfg.OUT + 2            # layer2 matmul width (h|s|d)

    with TileContext(nc) as tc:
        with (
            tc.tile_pool(name="dram", bufs=1, space="DRAM") as dpool,
            tc.tile_pool(name="const", bufs=1) as cpool,
            tc.tile_pool(name="work", bufs=3) as wpool,
            tc.tile_pool(name="gath", bufs=2) as gpool,
            tc.tile_pool(name="bc", bufs=1) as bcpool,
        ):
            nc.gpsimd.load_library(library_config.mlp)

            t1_shard = dpool.tile([SP, P], BF16)
            t1_full = dpool.tile([NT, P], BF16, addr_space="Shared")
            t2_shard = dpool.tile([SP, P], BF16)
            t2_full = dpool.tile([NT, P], BF16, addr_space="Shared")

            ident = cpool.tile([P, P], F32)
            make_identity(nc, ident[:])
            identb = cpool.tile([P, P], BF16)
            nc.vector.tensor_copy(out=identb[:], in_=ident[:])
            io32 = cpool.tile([P, P], I32)
            nc.gpsimd.iota(io32[:], pattern=[[1, P]], base=0, channel_multiplier=0)
            iorow = cpool.tile([P, P], BF16)
            nc.vector.tensor_copy(out=iorow[:], in_=io32[:])
            ic32 = cpool.tile([P, 1], I32)
            nc.gpsimd.iota(ic32[:], pattern=[[0, 1]], base=0, channel_multiplier=1)
            iocol = cpool.tile([P, 1], BF16)
            nc.vector.tensor_copy(out=iocol[:], in_=ic32[:])

            # per-slot dst-local ids, column layout [slot-in-chunk, chunk]
            dcol_all = cpool.tile([P, nch_pad], BF16)
            nc.sync.dma_start(out=dcol_all[:], in_=dval_ext[:, :], transpose=True)

            _pp0cm = tc.tile_pool(name="psum0", bufs=2, space="PSUM")
            ppool0 = _pp0cm.__enter__()

            # ---------- weight prep ----------
            w1_t = cpool.tile([cfg.IN, H * cfg.HID], F32)
            nc.sync.dma_start(out=w1_t[:], in_=W1_ext[:, :])
            w2_t = cpool.tile([H * cfg.HID, cfg.OUT], F32)
            nc.sync.dma_start(out=w2_t[:], in_=W2_ext[:, :])
            av = cpool.tile([cfg.HID, 2 * H + 2], F32)
            for h in range(H):
                nc.sync.dma_start(out=av[:, h:h + 1], in_=as1_ext[h:h + 1, :])
                nc.sync.dma_start(out=av[:, H + h:H + h + 1], in_=ad1_ext[h:h + 1, :])
            nc.sync.dma_start(out=av[:, 2 * H:2 * H + 1], in_=as2_ext[0:1, :])
            nc.sync.dma_start(out=av[:, 2 * H + 1:2 * H + 2], in_=ad2_ext[0:1, :])

            w1Th = cpool.tile([cfg.HID, H, cfg.IN], F32)
            for h in range(H):
                w1Th_p = ppool0.tile([cfg.HID, cfg.IN], F32, space="PSUM", tag="prep")
                nc.tensor.transpose(out=w1Th_p[:],
                                    in_=w1_t[:, h * cfg.HID:(h + 1) * cfg.HID],
                                    identity=ident[0:cfg.IN, 0:cfg.IN])
                nc.vector.tensor_copy(out=w1Th[:, h, :], in_=w1Th_p[:])
            w2T_p = ppool0.tile([cfg.OUT, H * cfg.HID], F32, space="PSUM", tag="prep")
            nc.tensor.transpose(out=w2T_p[:], in_=w2_t[:, :],
                                identity=ident[0:H * cfg.HID, 0:H * cfg.HID])
            w2T = cpool.tile([cfg.OUT, H * cfg.HID], F32)
            nc.vector.tensor_copy(out=w2T[:], in_=w2T_p[:])

            wv_p = ppool0.tile([cfg.IN, 2 * H + 2], F32, space="PSUM", tag="prep2")
            for h in range(H):
                nc.tensor.matmul(out=wv_p[:, h:h + 1],
                                 lhsT=w1Th[:, h, :],
                                 rhs=av[0:cfg.HID, h:h + 1], start=True, stop=True)
                nc.tensor.matmul(out=wv_p[:, H + h:H + h + 1],
                                 lhsT=w1Th[:, h, :],
                                 rhs=av[0:cfg.HID, H + h:H + h + 1], start=True, stop=True)
            nc.tensor.matmul(out=wv_p[0:H * cfg.HID, 2 * H:2 * H + 1], lhsT=w2T[:, :],
                             rhs=av[0:cfg.OUT, 2 * H:2 * H + 1], start=True, stop=True)
            nc.tensor.matmul(out=wv_p[0:H * cfg.HID, 2 * H + 1:2 * H + 2], lhsT=w2T[:, :],
                             rhs=av[0:cfg.OUT, 2 * H + 1:2 * H + 2], start=True, stop=True)

            # W1ext bf16 [IN, 70]: [W1h0 | 0 | W1h1 | 0 | s0 s1 d0 d1]
            w1e = cpool.tile([cfg.IN, NC1], BF16)
            for h in range(H):
                nc.vector.tensor_copy(out=w1e[:, h * (cfg.HID + 1):h * (cfg.HID + 1) + cfg.HID],
                                      in_=w1_t[:, h * cfg.HID:(h + 1) * cfg.HID])
                nc.vector.memset(w1e[:, h * (cfg.HID + 1) + cfg.HID:(h + 1) * (cfg.HID + 1)], 0.0)
            nc.vector.tensor_copy(out=w1e[:, SD1:SD1 + H], in_=wv_p[:, 0:H])
            nc.vector.tensor_copy(out=w1e[:, SD1 + H:NC1], in_=wv_p[:, H:2 * H])
            # W2ext f32 [64, 34]: [W2 | s2vec | d2vec]
            w2e = cpool.tile([H * cfg.HID, NC2], F32)
            nc.vector.tensor_copy(out=w2e[:, 0:cfg.OUT], in_=w2_t[:, :])
            nc.vector.tensor_copy(out=w2e[:, cfg.OUT:NC2],
                                  in_=wv_p[0:H * cfg.HID, 2 * H:2 * H + 2])

            b1_bc = cpool.tile([P, H, cfg.HID], F32)
            b1_row = cpool.tile([1, H * cfg.HID], F32)
            nc.sync.dma_start(out=b1_row[:], in_=b1_ext[:, :])
            nc.gpsimd.partition_broadcast(
                out_ap=b1_bc[:].rearrange("p h d -> p (h d)"), in_ap=b1_row[:])
            b2_bc = cpool.tile([P, cfg.OUT], F32)
            b2_row = cpool.tile([1, cfg.OUT], F32)
            nc.sync.dma_start(out=b2_row[:], in_=b2_ext[:, :])
            nc.gpsimd.partition_broadcast(out_ap=b2_bc[:], in_ap=b2_row[:])

            # ---------- persistent per-node state ----------
            own1 = cpool.tile([P, ST, P], BF16)    # own-shard table1 rows
            own2 = cpool.tile([P, ST, P], BF16)    # own-shard table2 rows
            g_all = cpool.tile([P, ST, H, cfg.HID], F32)
            d1o = cpool.tile([P, ST, H], BF16)
            d2o = cpool.tile([P, ST, 1], BF16)
            nc.vector.memset(own1[:].rearrange("p a b -> p (a b)"), 0.0)
            nc.vector.memset(own2[:].rearrange("p a b -> p (a b)"), 0.0)

            # ---------- phase 1: table1 shard build ----------
            t1_writes = []
            for st in range(ST):
                xf = wpool.tile([P, cfg.IN], BF16, tag="xf")
                nc.sync.dma_start(out=xf[:], in_=x_ext[st * P:(st + 1) * P, :])
                xT_p = ppool0.tile([cfg.IN, P], BF16, space="PSUM", tag="xT")
                nc.tensor.transpose(out=xT_p[:], in_=xf[:], identity=identb[:])
                xT = wpool.tile([cfg.IN, P], BF16, tag="xTs")
                nc.vector.tensor_copy(out=xT[:], in_=xT_p[:])
                hp = ppool0.tile([P, NC1], F32, space="PSUM", tag="hp")
                nc.tensor.matmul(out=hp[:, :], lhsT=xT[:], rhs=w1e[:, :],
                                 start=True, stop=True)
                nc.vector.tensor_copy(out=own1[:, st, 0:NC1], in_=hp[:, :])
                ones_view = own1[:, st, 0:SD1].rearrange(
                    "p (h d) -> p h d", h=H)[:, :, cfg.HID:cfg.HID + 1]
                nc.vector.memset(ones_view, 1.0)
                nc.vector.tensor_copy(out=d1o[:, st, :], in_=hp[:, SD1 + H:NC1])
                t1_writes.append(nc.sync.dma_start(
                    out=t1_shard[st * P:(st + 1) * P, :], in_=own1[:, st, :]).ins)

            cc1 = nc.gpsimd.collective_compute(
                "AllGather", mybir.AluOpType.bypass,
                ins=[t1_shard[:].opt()], outs=[t1_full[:].opt()],
                replica_groups=[list(range(cfg.CORES))])
            for w in t1_writes:
                add_dep_helper(cc1.ins, w, reason="t1 shard complete before AG")
            fences = {1: cc1.ins}

            _pp0cm.__exit__(None, None, None)
            _pp1cm = tc.tile_pool(name="psum_e", bufs=2, space="PSUM")
            ppool1 = _pp1cm.__enter__()
            _pp2cm = tc.tile_pool(name="psum_t", bufs=1, space="PSUM")
            ppool2 = _pp2cm.__enter__()

            # ---------- shared edge-pass ----------
            qn_counter = [0]

            def edge_pass(layer):
                if layer == 1:
                    table, heads, scol, own = t1_full, H, SD1, own1
                    mw = cfg.HID + 1
                    down = d1o
                else:
                    table, heads, scol, own = t2_full, 1, cfg.OUT + 1, own2
                    mw = cfg.OUT + 1
                    down = d2o
                for gi, g in enumerate(groups):
                    nchg, nb, ch0g = g["nchg"], g["nb"], g["ch0g"]
                    sts = g["sts"]
                    ns = len(sts)
                    S = nchg * P
                    # structure build
                    idx_t = gpool.tile([P, C16max], I16, tag="idx")
                    if nb:
                        for k in range(8):
                            nc.sync.dma_start(
                                out=idx_t[16 * k:16 * (k + 1), 0:nb * 8],
                                in_=idx_ext[gi, :, 0:nb * 8])
                    dbc = bcpool.tile([P, CHmax * P], BF16, tag="dbc")
                    nc.sync.dma_start(
                        out=dbc[:, 0:S],
                        in_=dval_ext[ch0g:ch0g + nchg, :]
                        .rearrange("a b -> (a b)")
                        .rearrange("(a s) -> a s", a=1).to_broadcast([P, S]))
                    m0 = gpool.tile([P, CHmax, P], FP8, tag="m0")
                    nc.vector.tensor_tensor(
                        out=m0[:, 0:nchg, :],
                        in0=dcol_all[:, ch0g:ch0g + nchg]
                        .rearrange("p (c a) -> p c a", a=1).to_broadcast([P, nchg, P]),
                        in1=iorow[:].rearrange("p (a b) -> p a b", a=1)
                        .to_broadcast([P, nchg, P]),
                        op=mybir.AluOpType.is_equal)
                    m0t = gpool.tile([P, CHmax * P], FP8, tag="m0t")
                    nc.vector.tensor_tensor(
                        out=m0t[:, 0:S], in0=dbc[:, 0:S],
                        in1=iocol[:].to_broadcast([P, S]),
                        op=mybir.AluOpType.is_equal)
                    # gather + self rows
                    gath = gpool.tile([P, CHmax, P], BF16, tag="gath")
                    for b in range(NB):
                        off16, nidx, ch0 = g["calls"][b]
                        while nidx > 0:
                            n = min(nidx, 4096)
                            qn = qn_counter[0] % 4
                            qn_counter[0] += 1
                            gi_inst = nc.gpsimd.dma_gather(
                                gath[:, ch0:ch0 + n // P, :],
                                table[b * BR:NT, :],
                                idx_t[:, off16:off16 + n // 16],
                                n, n, P, single_packet=False, queue_num=qn)
                            add_dep_helper(gi_inst.ins, fences[layer],
                                           reason="table ready before gather")
                            nidx -= n
                            ch0 += n // P
                            off16 += n // 16
                    for s in sts:
                        nc.vector.tensor_copy(
                            out=gath[:, g["self_ci"][s], :], in_=own[:, s, :])
                    # t = s_src + d_dst ; ex = exp(leakyrelu(t))
                    tp = ppool1.tile([P, CHmax, H], F32, space="PSUM", tag="tp")
                    for ci in range(nchg):
                        nc.tensor.matmul(
                            out=tp[:, ci, 0:heads],
                            lhsT=m0t[:, ci * P:(ci + 1) * P],
                            rhs=down[:, int(g["c2s"][ci]), 0:heads],
                            start=True, stop=True)
                    ts = wpool.tile([P, CHmax, H], F32, tag="ts")
                    ex = wpool.tile([P, CHmax, H], F32, tag="ex")
                    nc.vector.tensor_tensor(
                        out=ts[:, 0:nchg, 0:heads], in0=tp[:, 0:nchg, 0:heads],
                        in1=gath[:, 0:nchg, scol:scol + heads],
                        op=mybir.AluOpType.add)
                    nc.vector.tensor_scalar_mul(
                        out=ex[:, 0:nchg, 0:heads], in0=ts[:, 0:nchg, 0:heads],
                        scalar1=cfg.neg)
                    nc.vector.tensor_tensor(
                        out=ts[:, 0:nchg, 0:heads], in0=ts[:, 0:nchg, 0:heads],
                        in1=ex[:, 0:nchg, 0:heads], op=mybir.AluOpType.max)
                    nc.scalar.activation(
                        out=ex[:, 0:nchg, 0:heads], in_=ts[:, 0:nchg, 0:heads],
                        func=mybir.ActivationFunctionType.Exp)
                    for h in range(heads):
                        nc.vector.tensor_tensor(
                            out=gath[:, 0:nchg, h * mw:(h + 1) * mw],
                            in0=gath[:, 0:nchg, h * mw:(h + 1) * mw],
                            in1=ex[:, 0:nchg, h:h + 1].to_broadcast([P, nchg, mw]),
                            op=mybir.AluOpType.mult)
                    # aggregate per supertile
                    aggp = ppool1.tile([P, cfg.GST, heads, mw], F32,
                                       space="PSUM", tag="agg")
                    for sti, s in enumerate(sts):
                        lst = g["st_chunks"][s]
                        for j, ci in enumerate(lst):
                            nc.tensor.matmul(
                                out=aggp[:, sti, :, :].rearrange("p h m -> p (h m)"),
                                lhsT=m0[:, ci, :],
                                rhs=gath[:, ci, 0:heads * mw],
                                start=(j == 0), stop=(j == len(lst) - 1))
                    # normalize (+eps keeps pad rows finite)
                    rec = wpool.tile([P, cfg.GST, H, 1], F32, tag="rec")
                    nc.vector.tensor_scalar_add(
                        out=rec[:, 0:ns, 0:heads, :],
                        in0=aggp[:, 0:ns, :, mw - 1:mw], scalar1=1e-30)
                    nc.vector.reciprocal(out=rec[:, 0:ns, 0:heads, :],
                                         in_=rec[:, 0:ns, 0:heads, :])
                    st0 = sts[0]
                    if layer == 1:
                        gv = g_all[:, st0:st0 + ns, :, :]
                        nc.vector.tensor_tensor(
                            out=gv, in0=aggp[:, 0:ns, :, 0:cfg.HID],
                            in1=rec[:, 0:ns, :, :].to_broadcast([P, ns, H, cfg.HID]),
                            op=mybir.AluOpType.mult)
                        nc.vector.tensor_tensor(
                            out=gv, in0=gv,
                            in1=b1_bc[:].rearrange("p (a h) d -> p a h d", a=1)
                            .to_broadcast([P, ns, H, cfg.HID]),
                            op=mybir.AluOpType.add)
                        gvf = gv.rearrange("p a h d -> p (a h d)")
                        if cfg.sim_gelu:
                            _gelu_tanh(nc, wpool, gvf)
                        else:
                            nc.scalar.activation(
                                out=gvf, in_=gvf,
                                func=mybir.ActivationFunctionType.Gelu)
                    else:
                        ov = wpool.tile([P, cfg.GST, cfg.OUT], BF16, tag="ov")
                        nc.vector.tensor_tensor(
                            out=ov[:, 0:ns, :], in0=aggp[:, 0:ns, 0, 0:cfg.OUT],
                            in1=rec[:, 0:ns, 0, :].to_broadcast([P, ns, cfg.OUT]),
                            op=mybir.AluOpType.mult)
                        nc.vector.tensor_tensor(
                            out=ov[:, 0:ns, :], in0=ov[:, 0:ns, :],
                            in1=b2_bc[:].rearrange("p (a o) -> p a o", a=1)
                            .to_broadcast([P, ns, cfg.OUT]),
                            op=mybir.AluOpType.add)
                        for sti, s in enumerate(sts):
                            rows = min(P, cfg.SHARD - s * P)
                            nc.sync.dma_start(
                                out=out_ext[s * P:s * P + rows, :],
                                in_=ov[0:rows, sti, :])

            edge_pass(1)

            # ---------- phase: table2 shard build ----------
            t2_writes = []
            for st in range(ST):
                gT_p = ppool2.tile([H * cfg.HID, P], F32, space="PSUM", tag="gT")
                nc.tensor.transpose(
                    out=gT_p[:], in_=g_all[:, st, :, :].rearrange("p h d -> p (h d)"),
                    identity=ident[:])
                gT = wpool.tile([H * cfg.HID, P], F32, tag="gTs")
                nc.scalar.activation(out=gT[:], in_=gT_p[:],
                                     func=mybir.ActivationFunctionType.Copy)
                h2p = ppool2.tile([P, NC2], F32, space="PSUM", tag="h2p")
                nc.tensor.matmul(out=h2p[:], lhsT=gT[:], rhs=w2e[:, :],
                                 start=True, stop=True)
                nc.vector.tensor_copy(out=own2[:, st, 0:cfg.OUT], in_=h2p[:, 0:cfg.OUT])
                nc.vector.memset(own2[:, st, cfg.OUT:cfg.OUT + 1], 1.0)
                nc.vector.tensor_copy(out=own2[:, st, cfg.OUT + 1:cfg.OUT + 3],
                                      in_=h2p[:, cfg.OUT:NC2])
                nc.vector.tensor_copy(out=d2o[:, st, :], in_=h2p[:, NC2 - 1:NC2])
                t2_writes.append(nc.sync.dma_start(
                    out=t2_shard[st * P:(st + 1) * P, :], in_=own2[:, st, :]).ins)

            cc2 = nc.gpsimd.collective_compute(
                "AllGather", mybir.AluOpType.bypass,
                ins=[t2_shard[:].opt()], outs=[t2_full[:].opt()],
                replica_groups=[list(range(cfg.CORES))])
            for w in t2_writes:
                add_dep_helper(cc2.ins, w, reason="t2 shard complete before AG")
            fences[2] = cc2.ins

            edge_pass(2)
            _pp2cm.__exit__(None, None, None)
            _pp1cm.__exit__(None, None, None)

    nc.compile()
    return nc


def _gelu_tanh(nc, wpool, gv):
    """tanh-approx gelu in-place on gv [P, D] (CoreSim-compatible)."""
    Pp, D = gv.shape[0], gv.shape[1]
    t1 = wpool.tile([Pp, D], F32, tag="glu1")
    nc.scalar.activation(out=t1[:], in_=gv, scale=0.1888856,
                         func=mybir.ActivationFunctionType.Square)
    nc.vector.tensor_scalar_add(out=t1[:], in0=t1[:], scalar1=0.7978846)
    nc.vector.tensor_tensor(out=t1[:], in0=t1[:], in1=gv, op=mybir.AluOpType.mult)
    nc.scalar.activation(out=t1[:], in_=t1[:],
                         func=mybir.ActivationFunctionType.Tanh)
    nc.vector.tensor_scalar_add(out=t1[:], in0=t1[:], scalar1=1.0)
    nc.vector.tensor_tensor(out=t1[:], in0=t1[:], in1=gv, op=mybir.AluOpType.mult)
    nc.vector.tensor_scalar_mul(out=gv, in0=t1[:], scalar1=0.5)


_CACHE = {}


def _get_built(cfg, edge_index):
    key = hash((edge_index.tobytes(), cfg.N, cfg.E, cfg.GST, cfg.sim_gelu,
                cfg.BUCKET_ROWS, "v2"))
    if key not in _CACHE:
        sched, arrays = preprocess(edge_index, cfg)
        nc = build_nc(cfg, sched)
        _CACHE[key] = (nc, sched, arrays)
    return _CACHE[key]


def make_in_maps(cfg, sched, arrays, inputs):
    off, blob_len = blob_layout(cfg, sched)
    x = np.ascontiguousarray(inputs["x"], dtype=np.float32)

    def put(blob, name, arr):
        v = arr.reshape(-1).view(np.int16)
        blob[off[name]:off[name] + v.size] = v

    weights = {k: np.ascontiguousarray(inputs[k], dtype=np.float32)
               for k in ["W1", "a_src1", "a_dst1", "b1", "W2",
                         "a_src2", "a_dst2", "b2"]}
    in_maps = []
    for r in range(cfg.CORES):
        blob = np.zeros(blob_len, dtype=np.int16)
        xs = np.zeros((cfg.SHARD_PAD, cfg.IN), dtype=ml_dtypes.bfloat16)
        xs[0:cfg.SHARD] = x[r * cfg.SHARD:(r + 1) * cfg.SHARD]
        put(blob, "x", xs)
        put(blob, "dval", arrays[r]["dval"])
        put(blob, "idx", arrays[r]["idx"])
        for k, v in weights.items():
            put(blob, k, v)
        in_maps.append(dict(blob=blob.reshape(1, -1)))
    return in_maps


def kernel(x, edge_index, W1, a_src1, a_dst1, b1, W2, a_src2, a_dst2, b2,
           cfg=None, return_extras=False):
    from concourse.bass_utils import run_bass_kernel_spmd
    cfg = cfg or Cfg()
    nc, sched, arrays = _get_built(cfg, np.asarray(edge_index))
    in_maps = make_in_maps(cfg, sched, arrays, dict(
        x=x, W1=W1, a_src1=a_src1, a_dst1=a_dst1, b1=b1,
        W2=W2, a_src2=a_src2, a_dst2=a_dst2, b2=b2))
    res = run_bass_kernel_spmd(nc, in_maps, list(range(cfg.CORES)))
    out = np.concatenate(
        [np.asarray(res.results[r]["out"]).astype(np.float32)
         for r in range(cfg.CORES)], axis=0)
    if return_extras:
        return out, res
    return out


# revision 23
# speedup vs baseline: 1.3103x; 1.0861x over previous
"""Trainium2 Bass kernel for 2-layer GAT (nn_GAT_86535001080291).

Strategy (dst-sharded graph parallelism over 8 NeuronCores):
  - Core r owns destination nodes [r*12500, (r+1)*12500).
  - Per-node "table" rows (256B, bf16) hold per-node quantities:
      layer1: [h0(32) | 1 | h1(32) | 1 | s0 s1 d0 d1]
      layer2: [h2(32) | 1 | s2 | d2]
    Each core builds its own table shard from its x shard; shards are
    replicated via AllGather.
  - Edges (self-loops handled separately) are grouped per core into
    supertiles of 128 dst nodes; per (supertile, src-bucket) runs are
    padded to 128-edge chunks (buckets of <=25088 table rows keep
    dma_gather's int16 indices in range). Per-edge source rows are
    fetched with nc.gpsimd.dma_gather. Self-loop rows come from an
    SBUF-resident copy of the core's own table shard (one 128-row
    "self chunk" per supertile; no gather, no bucket inflation).
  - One-hot structure matrices are built ON DEVICE from a compact
    per-slot dst-local-id array (dval, bf16):
      m0 [slot, dst]  = (iota_row == dval_col)   for aggregation
      m0t [dst, slot] = (iota_col == dval_bc)    for d-expansion
    This keeps host->device input traffic tiny (the measured harness
    cost is dominated by per-iteration input shipping).
      t = s_src + d_dst        : d-expansion via PE matmul m0t.T @ d (+ s)
      ex = exp(leakyrelu(t))   : DVE + ACT (logits bounded -> exp safe)
      messages *= ex           : DVE broadcast multiply (incl. ones col)
      agg[dst]  = m0.T @ msgs  : PE matmul accumulating in PSUM; the
                                 ones-column yields the softmax denom
      out[dst]  = agg / denom  (+ bias, gelu between layers)
All host-side preprocessing depends only on edge_index (graph structure).
"""
import math
from dataclasses import dataclass

import numpy as np
import ml_dtypes

import concourse.bacc as bacc
import concourse.mybir as mybir
from concourse.tile import TileContext
from concourse.masks import make_identity
from concourse.tile_rust import add_dep_helper
from concourse import library_config

F32 = mybir.dt.float32
BF16 = mybir.dt.bfloat16
FP8 = mybir.dt.float8e4
I16 = mybir.dt.int16
I32 = mybir.dt.int32
P = 128


@dataclass
class Cfg:
    N: int = 100000
    E: int = 1600000
    IN: int = 64
    HID: int = 32
    HEADS: int = 2
    OUT: int = 32
    neg: float = 0.2
    CORES: int = 8
    GST: int = 4             # supertiles per group
    BUCKET_ROWS: int = 25088  # int16 gather index limit
    sim_gelu: bool = False   # tanh-approx gelu (CoreSim lacks Gelu LUT)

    @property
    def SHARD(self):
        return self.N // self.CORES

    @property
    def ST(self):
        return math.ceil(self.SHARD / P)

    @property
    def SHARD_PAD(self):
        return self.ST * P

    @property
    def NT(self):
        return self.CORES * self.SHARD_PAD

    @property
    def NBUCK(self):
        return max(1, math.ceil(self.NT / self.BUCKET_ROWS))

    @property
    def BROWS(self):
        return (self.NT + self.NBUCK - 1) // self.NBUCK


def build_schedule(cfg, B_sb):
    """Shared (core-independent) static schedule from padded chunk counts.

    Per group: slot layout is bucket-major over the group's supertiles,
    followed by one full 128-slot self chunk per supertile.
    """
    groups = []
    ch0g = 0
    st = 0
    while st < cfg.ST:
        sts = list(range(st, min(st + cfg.GST, cfg.ST)))
        g = dict(sts=sts, ch0g=ch0g)
        ci = 0
        calls = []
        runs = {}
        for b in range(cfg.NBUCK):
            off16 = ci * 8
            ch0b = ci
            for s in sts:
                B = int(B_sb[s][b])
                runs[(s, b)] = (ci, B)
                ci += B
            calls.append((off16, (ci - ch0b) * P, ch0b))
        g["nb"] = ci                      # bucket chunks
        g["self_ci"] = {s: ci + i for i, s in enumerate(sts)}
        ci += len(sts)
        g["nchg"] = ci
        g["calls"] = calls
        g["runs"] = runs
        # chunk -> st map (self chunks included)
        c2s = np.zeros(ci, dtype=np.int64)
        for (s, b), (c0, B) in runs.items():
            c2s[c0:c0 + B] = s
        for s, c in g["self_ci"].items():
            c2s[c] = s
        g["c2s"] = c2s
        # st-major chunk order for aggregation (self chunk last)
        order = {}
        for s in sts:
            lst = []
            for b in range(cfg.NBUCK):
                c0, B = runs[(s, b)]
                lst.extend(range(c0, c0 + B))
            lst.append(g["self_ci"][s])
            order[s] = lst
        g["st_chunks"] = order
        ch0g += ci
        groups.append(g)
        st += cfg.GST
    nch_total = ch0g
    nch_pad = (nch_total + 15) // 16 * 16
    return groups, nch_total, nch_pad


def preprocess(edge_index, cfg):
    """Pure graph preprocessing: per-core gather indices + dst-id slots."""
    src = edge_index[0].astype(np.int64)
    dst = edge_index[1].astype(np.int64)

    SH, SP, ST, NB, BR = cfg.SHARD, cfg.SHARD_PAD, cfg.ST, cfg.NBUCK, cfg.BROWS

    per_core = []
    cnt = np.zeros((cfg.CORES, ST, NB), dtype=np.int64)
    for r in range(cfg.CORES):
        m = (dst >= r * SH) & (dst < (r + 1) * SH)
        s_r = src[m]
        d_r = dst[m] - r * SH
        srow = (s_r // SH) * SP + (s_r % SH)
        b_r = srow // BR
        st_r = d_r // P
        per_core.append((srow, d_r, b_r, st_r))
        np.add.at(cnt[r], (st_r, b_r), 1)

    B_sb = np.ceil(cnt.max(axis=0) / P).astype(np.int64)  # [ST, NB]
    groups, nch_total, nch_pad = build_schedule(cfg, B_sb)
    NG = len(groups)
    CHmax = max(g["nchg"] for g in groups)
    C16max = max(g["nb"] for g in groups) * 8

    # lookup tables: (st, b) -> group idx, group-relative chunk offset
    gi_tab = np.zeros((ST, NB), np.int64)
    ch0_tab = np.zeros((ST, NB), np.int64)
    ch0g_tab = np.zeros(ST, np.int64)
    for gi, g in enumerate(groups):
        for (s, b), (c0, B) in g["runs"].items():
            gi_tab[s, b] = gi
            ch0_tab[s, b] = c0
        for s in g["sts"]:
            ch0g_tab[s] = g["ch0g"]

    arrays = []
    for r in range(cfg.CORES):
        srow, d_r, b_r, st_r = per_core[r]
        # sort edges by (st, b) cell, compute rank within cell
        cell_key = st_r * NB + b_r
        order = np.argsort(cell_key, kind="stable")
        srow, d_r, b_r, st_r = srow[order], d_r[order], b_r[order], st_r[order]
        cell_key = cell_key[order]
        ne = len(cell_key)
        if ne:
            change = np.empty(ne, dtype=bool)
            change[0] = True
            change[1:] = cell_key[1:] != cell_key[:-1]
            starts = np.flatnonzero(change)
            rank = np.arange(ne) - np.repeat(
                starts, np.diff(np.append(starts, ne)))
        else:
            rank = np.zeros(0, np.int64)

        gi_e = gi_tab[st_r, b_r]
        slot = ch0_tab[st_r, b_r] * P + rank     # group-relative slot

        idx16 = np.zeros((NG, 16, C16max), dtype=np.int16)
        idx16[gi_e, slot % 16, slot // 16] = (srow - b_r * BR).astype(np.int16)

        dval8 = np.full((nch_pad, P), 255, dtype=np.uint8)
        gslot = (ch0g_tab[st_r] * P + slot)      # global slot
        dval8[gslot // P, gslot % P] = (d_r % P).astype(np.uint8)
        # self chunks: dloc = iota
        for g in groups:
            for s, c in g["self_ci"].items():
                dval8[g["ch0g"] + c, :] = np.arange(P, dtype=np.uint8)

        arrays.append(dict(
            idx=idx16,
            dval=dval8.astype(ml_dtypes.bfloat16),
        ))

    sched = dict(groups=groups, NG=NG, CHmax=CHmax, C16max=C16max,
                 nch_total=nch_total, nch_pad=nch_pad)
    return sched, arrays


def blob_layout(cfg, sched):
    """Byte layout (in int16 units) of the single per-core input blob."""
    NG, C16max = sched["NG"], sched["C16max"]
    nch_pad = sched["nch_pad"]
    H = cfg.HEADS
    off = {}
    o = 0
    off["x"] = o
    o += cfg.SHARD_PAD * cfg.IN                 # bf16
    off["dval"] = o
    o += nch_pad * P                            # bf16
    off["idx"] = o
    o += NG * 16 * C16max                       # i16
    # f32 weights (2 i16 units each)
    for name, n in [("W1", cfg.IN * H * cfg.HID), ("a_src1", H * cfg.HID),
                    ("a_dst1", H * cfg.HID), ("b1", H * cfg.HID),
                    ("W2", H * cfg.HID * cfg.OUT), ("a_src2", cfg.OUT),
                    ("a_dst2", cfg.OUT), ("b2", cfg.OUT)]:
        off[name] = o
        o += 2 * n
    return off, o


def build_nc(cfg, sched):
    nc = bacc.Bacc("TRN2", target_bir_lowering=False, num_swdge_queues=4)
    NG, CHmax, C16max = sched["NG"], sched["CHmax"], sched["C16max"]
    nch_pad = sched["nch_pad"]
    groups = sched["groups"]
    ST, NT, SP, NB, BR = cfg.ST, cfg.NT, cfg.SHARD_PAD, cfg.NBUCK, cfg.BROWS
    H = cfg.HEADS

    # ---- external I/O (single packed input blob) ----
    off, blob_len = blob_layout(cfg, sched)
    blob_ext = nc.dram_tensor("blob", [1, blob_len], I16, kind="ExternalInput")
    out_ext = nc.dram_tensor("out", [cfg.SHARD, cfg.OUT], BF16, kind="ExternalOutput")

    def bview(name, n_i16, dt, rows=None):
        v = blob_ext[0, off[name]:off[name] + n_i16]
        if dt != I16:
            v = v.bitcast(dt)
        if rows is not None:
            v = v.rearrange("(a b) -> a b", a=rows)
        return v

    x_ext = bview("x", SP * cfg.IN, BF16, rows=SP)
    dval_ext = bview("dval", nch_pad * P, BF16, rows=nch_pad)
    idx_ext = bview("idx", NG * 16 * C16max, I16).rearrange(
        "(g p c) -> g p c", g=NG, p=16)
    W1_ext = bview("W1", 2 * cfg.IN * H * cfg.HID, F32, rows=cfg.IN)
    as1_ext = bview("a_src1", 2 * H * cfg.HID, F32, rows=H)
    ad1_ext = bview("a_dst1", 2 * H * cfg.HID, F32, rows=H)
    b1_ext = bview("b1", 2 * H * cfg.HID, F32, rows=1)
    W2_ext = bview("W2", 2 * H * cfg.HID * cfg.OUT, F32, rows=H * cfg.HID)
    as2_ext = bview("a_src2", 2 * cfg.OUT, F32, rows=1)
    ad2_ext = bview("a_dst2", 2 * cfg.OUT, F32, rows=1)
    b2_ext = bview("b2", 2 * cfg.OUT, F32, rows=1)

    SD1 = 2 * (cfg.HID + 1)      # layer1 s-col offset (=66)
    NC1 = SD1 + 2 * H            # layer1 packed width (=70)
    NC2 = cfg.OUT + 2            # layer2 matmul width (h|s|d)

    with TileContext(nc) as tc:
        with (
            tc.tile_pool(name="dram", bufs=1, space="DRAM") as dpool,
            tc.tile_pool(name="const", bufs=1) as cpool,
            tc.tile_pool(name="work", bufs=3) as wpool,
            tc.tile_pool(name="gath", bufs=2) as gpool,
            tc.tile_pool(name="bc", bufs=1) as bcpool,
        ):
            nc.gpsimd.load_library(library_config.mlp)

            t1_shard = dpool.tile([SP, P], BF16)
            t1_full = dpool.tile([NT, P], BF16, addr_space="Shared")
            t2_shard = dpool.tile([SP, P], BF16)
            t2_full = dpool.tile([NT, P], BF16, addr_space="Shared")

            ident = cpool.tile([P, P], F32)
            make_identity(nc, ident[:])
            identb = cpool.tile([P, P], BF16)
            nc.vector.tensor_copy(out=identb[:], in_=ident[:])
            io32 = cpool.tile([P, P], I32)
            nc.gpsimd.iota(io32[:], pattern=[[1, P]], base=0, channel_multiplier=0)
            iorow = cpool.tile([P, P], BF16)
            nc.vector.tensor_copy(out=iorow[:], in_=io32[:])
            ic32 = cpool.tile([P, 1], I32)
            nc.gpsimd.iota(ic32[:], pattern=[[0, 1]], base=0, channel_multiplier=1)
            iocol = cpool.tile([P, 1], BF16)
            nc.vector.tensor_copy(out=iocol[:], in_=ic32[:])

            # per-slot dst-local ids, column layout [slot-in-chunk, chunk]
            dcol_all = cpool.tile([P, nch_pad], BF16)
            nc.sync.dma_start(out=dcol_all[:], in_=dval_ext[:, :], transpose=True)

            _pp0cm = tc.tile_pool(name="psum0", bufs=2, space="PSUM")
            ppool0 = _pp0cm.__enter__()

            # ---------- weight prep ----------
            w1_t = cpool.tile([cfg.IN, H * cfg.HID], F32)
            nc.sync.dma_start(out=w1_t[:], in_=W1_ext[:, :])
            w2_t = cpool.tile([H * cfg.HID, cfg.OUT], F32)
            nc.sync.dma_start(out=w2_t[:], in_=W2_ext[:, :])
            av = cpool.tile([cfg.HID, 2 * H + 2], F32)
            for h in range(H):
                nc.sync.dma_start(out=av[:, h:h + 1], in_=as1_ext[h:h + 1, :])
                nc.sync.dma_start(out=av[:, H + h:H + h + 1], in_=ad1_ext[h:h + 1, :])
            nc.sync.dma_start(out=av[:, 2 * H:2 * H + 1], in_=as2_ext[0:1, :])
            nc.sync.dma_start(out=av[:, 2 * H + 1:2 * H + 2], in_=ad2_ext[0:1, :])

            w1Th = cpool.tile([cfg.HID, H, cfg.IN], F32)
            for h in range(H):
                w1Th_p = ppool0.tile([cfg.HID, cfg.IN], F32, space="PSUM", tag="prep")
                nc.tensor.transpose(out=w1Th_p[:],
                                    in_=w1_t[:, h * cfg.HID:(h + 1) * cfg.HID],
                                    identity=ident[0:cfg.IN, 0:cfg.IN])
                nc.vector.tensor_copy(out=w1Th[:, h, :], in_=w1Th_p[:])
            w2T_p = ppool0.tile([cfg.OUT, H * cfg.HID], F32, space="PSUM", tag="prep")
            nc.tensor.transpose(out=w2T_p[:], in_=w2_t[:, :],
                                identity=ident[0:H * cfg.HID, 0:H * cfg.HID])
            w2T = cpool.tile([cfg.OUT, H * cfg.HID], F32)
            nc.vector.tensor_copy(out=w2T[:], in_=w2T_p[:])

            wv_p = ppool0.tile([cfg.IN, 2 * H + 2], F32, space="PSUM", tag="prep2")
            for h in range(H):
                nc.tensor.matmul(out=wv_p[:, h:h + 1],
                                 lhsT=w1Th[:, h, :],
                                 rhs=av[0:cfg.HID, h:h + 1], start=True, stop=True)
                nc.tensor.matmul(out=wv_p[:, H + h:H + h + 1],
                                 lhsT=w1Th[:, h, :],
                                 rhs=av[0:cfg.HID, H + h:H + h + 1], start=True, stop=True)
            nc.tensor.matmul(out=wv_p[0:H * cfg.HID, 2 * H:2 * H + 1], lhsT=w2T[:, :],
                             rhs=av[0:cfg.OUT, 2 * H:2 * H + 1], start=True, stop=True)
            nc.tensor.matmul(out=wv_p[0:H * cfg.HID, 2 * H + 1:2 * H + 2], lhsT=w2T[:, :],
                             rhs=av[0:cfg.OUT, 2 * H + 1:2 * H + 2], start=True, stop=True)

            # W1ext bf16 [IN, 70]: [W1h0 | 0 | W1h1 | 0 | s0 s1 d0 d1]
            w1e = cpool.tile([cfg.IN, NC1], BF16)
            for h in range(H):
                nc.vector.tensor_copy(out=w1e[:, h * (cfg.HID + 1):h * (cfg.HID + 1) + cfg.HID],
                                      in_=w1_t[:, h * cfg.HID:(h + 1) * cfg.HID])
                nc.vector.memset(w1e[:, h * (cfg.HID + 1) + cfg.HID:(h + 1) * (cfg.HID + 1)], 0.0)
            nc.vector.tensor_copy(out=w1e[:, SD1:SD1 + H], in_=wv_p[:, 0:H])
            nc.vector.tensor_copy(out=w1e[:, SD1 + H:NC1], in_=wv_p[:, H:2 * H])
            # W2ext f32 [64, 34]: [W2 | s2vec | d2vec]
            w2e = cpool.tile([H * cfg.HID, NC2], F32)
            nc.vector.tensor_copy(out=w2e[:, 0:cfg.OUT], in_=w2_t[:, :])
            nc.vector.tensor_copy(out=w2e[:, cfg.OUT:NC2],
                                  in_=wv_p[0:H * cfg.HID, 2 * H:2 * H + 2])

            b1_bc = cpool.tile([P, H, cfg.HID], F32)
            b1_row = cpool.tile([1, H * cfg.HID], F32)
            nc.sync.dma_start(out=b1_row[:], in_=b1_ext[:, :])
            nc.gpsimd.partition_broadcast(
                out_ap=b1_bc[:].rearrange("p h d -> p (h d)"), in_ap=b1_row[:])
            b2_bc = cpool.tile([P, cfg.OUT], F32)
            b2_row = cpool.tile([1, cfg.OUT], F32)
            nc.sync.dma_start(out=b2_row[:], in_=b2_ext[:, :])
            nc.gpsimd.partition_broadcast(out_ap=b2_bc[:], in_ap=b2_row[:])

            # ---------- persistent per-node state ----------
            own1 = cpool.tile([P, ST, P], BF16)    # own-shard table1 rows
            own2 = cpool.tile([P, ST, P], BF16)    # own-shard table2 rows
            g_all = cpool.tile([P, ST, H, cfg.HID], F32)
            d1o = cpool.tile([P, ST, H], BF16)
            d2o = cpool.tile([P, ST, 1], BF16)
            nc.vector.memset(own1[:].rearrange("p a b -> p (a b)"), 0.0)
            nc.vector.memset(own2[:].rearrange("p a b -> p (a b)"), 0.0)

            # ---------- phase 1: table1 shard build ----------
            t1_writes = []
            for st in range(ST):
                xf = wpool.tile([P, cfg.IN], BF16, tag="xf")
                nc.sync.dma_start(out=xf[:], in_=x_ext[st * P:(st + 1) * P, :])
                xT_p = ppool0.tile([cfg.IN, P], BF16, space="PSUM", tag="xT")
                nc.tensor.transpose(out=xT_p[:], in_=xf[:], identity=identb[:])
                xT = wpool.tile([cfg.IN, P], BF16, tag="xTs")
                nc.vector.tensor_copy(out=xT[:], in_=xT_p[:])
                hp = ppool0.tile([P, NC1], F32, space="PSUM", tag="hp")
                nc.tensor.matmul(out=hp[:, :], lhsT=xT[:], rhs=w1e[:, :],
                                 start=True, stop=True)
                nc.vector.tensor_copy(out=own1[:, st, 0:NC1], in_=hp[:, :])
                ones_view = own1[:, st, 0:SD1].rearrange(
                    "p (h d) -> p h d", h=H)[:, :, cfg.HID:cfg.HID + 1]
                nc.vector.memset(ones_view, 1.0)
                nc.vector.tensor_copy(out=d1o[:, st, :], in_=hp[:, SD1 + H:NC1])
                t1_writes.append(nc.sync.dma_start(
                    out=t1_shard[st * P:(st + 1) * P, :], in_=own1[:, st, :]).ins)

            cc1 = nc.gpsimd.collective_compute(
                "AllGather", mybir.AluOpType.bypass,
                ins=[t1_shard[:].opt()], outs=[t1_full[:].opt()],
                replica_groups=[list(range(cfg.CORES))])
            for w in t1_writes:
                add_dep_helper(cc1.ins, w, reason="t1 shard complete before AG")
            fences = {1: cc1.ins}

            _pp0cm.__exit__(None, None, None)
            _pp1cm = tc.tile_pool(name="psum_e", bufs=2, space="PSUM")
            ppool1 = _pp1cm.__enter__()
            _pp2cm = tc.tile_pool(name="psum_t", bufs=1, space="PSUM")
            ppool2 = _pp2cm.__enter__()

            # ---------- shared edge-pass ----------
            qn_counter = [0]

            def edge_pass(layer):
                if layer == 1:
                    table, heads, scol, own = t1_full, H, SD1, own1
                    mw = cfg.HID + 1
                    down = d1o
                else:
                    table, heads, scol, own = t2_full, 1, cfg.OUT + 1, own2
                    mw = cfg.OUT + 1
                    down = d2o
                for gi, g in enumerate(groups):
                    nchg, nb, ch0g = g["nchg"], g["nb"], g["ch0g"]
                    sts = g["sts"]
                    ns = len(sts)
                    S = nchg * P
                    # structure build
                    idx_t = gpool.tile([P, C16max], I16, tag="idx")
                    if nb:
                        for k in range(8):
                            nc.sync.dma_start(
                                out=idx_t[16 * k:16 * (k + 1), 0:nb * 8],
                                in_=idx_ext[gi, :, 0:nb * 8])
                    dbc = bcpool.tile([P, CHmax * P], BF16, tag="dbc")
                    nc.sync.dma_start(
                        out=dbc[:, 0:S],
                        in_=dval_ext[ch0g:ch0g + nchg, :]
                        .rearrange("a b -> (a b)")
                        .rearrange("(a s) -> a s", a=1).to_broadcast([P, S]))
                    m0 = gpool.tile([P, CHmax, P], FP8, tag="m0")
                    nc.vector.tensor_tensor(
                        out=m0[:, 0:nchg, :],
                        in0=dcol_all[:, ch0g:ch0g + nchg]
                        .rearrange("p (c a) -> p c a", a=1).to_broadcast([P, nchg, P]),
                        in1=iorow[:].rearrange("p (a b) -> p a b", a=1)
                        .to_broadcast([P, nchg, P]),
                        op=mybir.AluOpType.is_equal)
                    m0t = gpool.tile([P, CHmax * P], FP8, tag="m0t")
                    nc.vector.tensor_tensor(
                        out=m0t[:, 0:S], in0=dbc[:, 0:S],
                        in1=iocol[:].to_broadcast([P, S]),
                        op=mybir.AluOpType.is_equal)
                    # gather + self rows
                    gath = gpool.tile([P, CHmax, P], BF16, tag="gath")
                    for b in range(NB):
                        off16, nidx, ch0 = g["calls"][b]
                        while nidx > 0:
                            n = min(nidx, 4096)
                            qn = qn_counter[0] % 4
                            qn_counter[0] += 1
                            gi_inst = nc.gpsimd.dma_gather(
                                gath[:, ch0:ch0 + n // P, :],
                                table[b * BR:NT, :],
                                idx_t[:, off16:off16 + n // 16],
                                n, n, P, single_packet=False, queue_num=qn)
                            add_dep_helper(gi_inst.ins, fences[layer],
                                           reason="table ready before gather")
                            nidx -= n
                            ch0 += n // P
                            off16 += n // 16
                    for s in sts:
                        nc.vector.tensor_copy(
                            out=gath[:, g["self_ci"][s], :], in_=own[:, s, :])
                    # t = s_src + d_dst ; ex = exp(leakyrelu(t))
                    tp = ppool1.tile([P, CHmax, H], F32, space="PSUM", tag="tp")
                    for ci in range(nchg):
                        nc.tensor.matmul(
                            out=tp[:, ci, 0:heads],
                            lhsT=m0t[:, ci * P:(ci + 1) * P],
                            rhs=down[:, int(g["c2s"][ci]), 0:heads],
                            start=True, stop=True)
                    ts = wpool.tile([P, CHmax, H], F32, tag="ts")
                    ex = wpool.tile([P, CHmax, H], F32, tag="ex")
                    nc.vector.tensor_tensor(
                        out=ts[:, 0:nchg, 0:heads], in0=tp[:, 0:nchg, 0:heads],
                        in1=gath[:, 0:nchg, scol:scol + heads],
                        op=mybir.AluOpType.add)
                    nc.vector.tensor_scalar_mul(
                        out=ex[:, 0:nchg, 0:heads], in0=ts[:, 0:nchg, 0:heads],
                        scalar1=cfg.neg)
                    nc.vector.tensor_tensor(
                        out=ts[:, 0:nchg, 0:heads], in0=ts[:, 0:nchg, 0:heads],
                        in1=ex[:, 0:nchg, 0:heads], op=mybir.AluOpType.max)
                    nc.scalar.activation(
                        out=ex[:, 0:nchg, 0:heads], in_=ts[:, 0:nchg, 0:heads],
                        func=mybir.ActivationFunctionType.Exp)
                    for h in range(heads):
                        nc.vector.tensor_tensor(
                            out=gath[:, 0:nchg, h * mw:(h + 1) * mw],
                            in0=gath[:, 0:nchg, h * mw:(h + 1) * mw],
                            in1=ex[:, 0:nchg, h:h + 1].to_broadcast([P, nchg, mw]),
                            op=mybir.AluOpType.mult)
                    # aggregate per supertile
                    aggp = ppool1.tile([P, cfg.GST, heads, mw], F32,
                                       space="PSUM", tag="agg")
                    for sti, s in enumerate(sts):
                        lst = g["st_chunks"][s]
                        for j, ci in enumerate(lst):
                            nc.tensor.matmul(
                                out=aggp[:, sti, :, :].rearrange("p h m -> p (h m)"),
                                lhsT=m0[:, ci, :],
                                rhs=gath[:, ci, 0:heads * mw],
                                start=(j == 0), stop=(j == len(lst) - 1))
                    # normalize (+eps keeps pad rows finite)
                    rec = wpool.tile([P, cfg.GST, H, 1], F32, tag="rec")
                    nc.vector.tensor_scalar_add(
                        out=rec[:, 0:ns, 0:heads, :],
                        in0=aggp[:, 0:ns, :, mw - 1:mw], scalar1=1e-30)
                    nc.vector.reciprocal(out=rec[:, 0:ns, 0:heads, :],
                                         in_=rec[:, 0:ns, 0:heads, :])
                    st0 = sts[0]
                    if layer == 1:
                        gv = g_all[:, st0:st0 + ns, :, :]
                        nc.vector.tensor_tensor(
                            out=gv, in0=aggp[:, 0:ns, :, 0:cfg.HID],
                            in1=rec[:, 0:ns, :, :].to_broadcast([P, ns, H, cfg.HID]),
                            op=mybir.AluOpType.mult)
                        nc.vector.tensor_tensor(
                            out=gv, in0=gv,
                            in1=b1_bc[:].rearrange("p (a h) d -> p a h d", a=1)
                            .to_broadcast([P, ns, H, cfg.HID]),
                            op=mybir.AluOpType.add)
                        gvf = gv.rearrange("p a h d -> p (a h d)")
                        if cfg.sim_gelu:
                            _gelu_tanh(nc, wpool, gvf)
                        else:
                            nc.scalar.activation(
                                out=gvf, in_=gvf,
                                func=mybir.ActivationFunctionType.Gelu)
                    else:
                        ov = wpool.tile([P, cfg.GST, cfg.OUT], BF16, tag="ov")
                        nc.vector.tensor_tensor(
                            out=ov[:, 0:ns, :], in0=aggp[:, 0:ns, 0, 0:cfg.OUT],
                            in1=rec[:, 0:ns, 0, :].to_broadcast([P, ns, cfg.OUT]),
                            op=mybir.AluOpType.mult)
                        nc.vector.tensor_tensor(
                            out=ov[:, 0:ns, :], in0=ov[:, 0:ns, :],
                            in1=b2_bc[:].rearrange("p (a o) -> p a o", a=1)
                            .to_broadcast([P, ns, cfg.OUT]),
                            op=mybir.AluOpType.add)
                        for sti, s in enumerate(sts):
                            rows = min(P, cfg.SHARD - s * P)
                            nc.sync.dma_start(
                                out=out_ext[s * P:s * P + rows, :],
                                in_=ov[0:rows, sti, :])

            edge_pass(1)

            # ---------- phase: table2 shard build ----------
            t2_writes = []
            for st in range(ST):
                gT_p = ppool2.tile([H * cfg.HID, P], F32, space="PSUM", tag="gT")
                nc.tensor.transpose(
                    out=gT_p[:], in_=g_all[:, st, :, :].rearrange("p h d -> p (h d)"),
                    identity=ident[:])
                gT = wpool.tile([H * cfg.HID, P], F32, tag="gTs")
                nc.scalar.activation(out=gT[:], in_=gT_p[:],
                                     func=mybir.ActivationFunctionType.Copy)
                h2p = ppool2.tile([P, NC2], F32, space="PSUM", tag="h2p")
                nc.tensor.matmul(out=h2p[:], lhsT=gT[:], rhs=w2e[:, :],
                                 start=True, stop=True)
                nc.vector.tensor_copy(out=own2[:, st, 0:cfg.OUT], in_=h2p[:, 0:cfg.OUT])
                nc.vector.memset(own2[:, st, cfg.OUT:cfg.OUT + 1], 1.0)
                nc.vector.tensor_copy(out=own2[:, st, cfg.OUT + 1:cfg.OUT + 3],
                                      in_=h2p[:, cfg.OUT:NC2])
                nc.vector.tensor_copy(out=d2o[:, st, :], in_=h2p[:, NC2 - 1:NC2])
                t2_writes.append(nc.sync.dma_start(
                    out=t2_shard[st * P:(st + 1) * P, :], in_=own2[:, st, :]).ins)

            cc2 = nc.gpsimd.collective_compute(
                "AllGather", mybir.AluOpType.bypass,
                ins=[t2_shard[:].opt()], outs=[t2_full[:].opt()],
                replica_groups=[list(range(cfg.CORES))])
            for w in t2_writes:
                add_dep_helper(cc2.ins, w, reason="t2 shard complete before AG")
            fences[2] = cc2.ins

            edge_pass(2)
            _pp2cm.__exit__(None, None, None)
            _pp1cm.__exit__(None, None, None)

    nc.compile()
    return nc


def _gelu_tanh(nc, wpool, gv):
    """tanh-approx gelu in-place on gv [P, D] (CoreSim-compatible)."""
    Pp, D = gv.shape[0], gv.shape[1]
    t1 = wpool.tile([Pp, D], F32, tag="glu1")
    nc.scalar.activation(out=t1[:], in_=gv, scale=0.1888856,
                         func=mybir.ActivationFunctionType.Square)
    nc.vector.tensor_scalar_add(out=t1[:], in0=t1[:], scalar1=0.7978846)
    nc.vector.tensor_tensor(out=t1[:], in0=t1[:], in1=gv, op=mybir.AluOpType.mult)
    nc.scalar.activation(out=t1[:], in_=t1[:],
                         func=mybir.ActivationFunctionType.Tanh)
    nc.vector.tensor_scalar_add(out=t1[:], in0=t1[:], scalar1=1.0)
    nc.vector.tensor_tensor(out=t1[:], in0=t1[:], in1=gv, op=mybir.AluOpType.mult)
    nc.vector.tensor_scalar_mul(out=gv, in0=t1[:], scalar1=0.5)


_CACHE = {}


def _get_built(cfg, edge_index):
    key = hash((edge_index.tobytes(), cfg.N, cfg.E, cfg.GST, cfg.sim_gelu,
                cfg.BUCKET_ROWS, "v2"))
    if key not in _CACHE:
        sched, arrays = preprocess(edge_index, cfg)
        nc = build_nc(cfg, sched)
        _CACHE[key] = (nc, sched, arrays)
    return _CACHE[key]


def make_in_maps(cfg, sched, arrays, inputs):
    off, blob_len = blob_layout(cfg, sched)
    x = np.ascontiguousarray(inputs["x"], dtype=np.float32)

    def put(blob, name, arr):
        v = arr.reshape(-1).view(np.int16)
        blob[off[name]:off[name] + v.size] = v

    weights = {k: np.ascontiguousarray(inputs[k], dtype=np.float32)
               for k in ["W1", "a_src1", "a_dst1", "b1", "W2",
                         "a_src2", "a_dst2", "b2"]}
    in_maps = []
    for r in range(cfg.CORES):
        blob = np.zeros(blob_len, dtype=np.int16)
        xs = np.zeros((cfg.SHARD_PAD, cfg.IN), dtype=ml_dtypes.bfloat16)
        xs[0:cfg.SHARD] = x[r * cfg.SHARD:(r + 1) * cfg.SHARD]
        put(blob, "x", xs)
        put(blob, "dval", arrays[r]["dval"])
        put(blob, "idx", arrays[r]["idx"])
        for k, v in weights.items():
            put(blob, k, v)
        in_maps.append(dict(blob=blob.reshape(1, -1)))
    return in_maps


def kernel(x, edge_index, W1, a_src1, a_dst1, b1, W2, a_src2, a_dst2, b2,
           cfg=None, return_extras=False):
    from concourse.bass_utils import run_bass_kernel_spmd
    cfg = cfg or Cfg()
    nc, sched, arrays = _get_built(cfg, np.asarray(edge_index))
    in_maps = make_in_maps(cfg, sched, arrays, dict(
        x=x, W1=W1, a_src1=a_src1, a_dst1=a_dst1, b1=b1,
        W2=W2, a_src2=a_src2, a_dst2=a_dst2, b2=b2))
    res = run_bass_kernel_spmd(nc, in_maps, list(range(cfg.CORES)))
    out = np.concatenate(
        [np.asarray(res.results[r]["out"]).astype(np.float32)
         for r in range(cfg.CORES)], axis=0)
    if return_extras:
        return out, res
    return out
